# revision 3
# baseline (speedup 1.0000x reference)
"""Trainium2 Bass kernel for a 2-layer multi-head GAT (gnn_message_passing).

Strategy (8 NeuronCores, SPMD), v2:
  - Nodes padded to NP = ceil(N/1024)*1024, split into 8 contiguous shards.
    Edges are assigned to the core that owns their SRC node, sorted/grouped
    by 128-node tile; each node tile's edge list is padded to ET edge tiles
    of 128 (static SPMD program; pad slots carry src_local=-1 and are
    SKIPPED by the gather via bounds_check, contributing zero through the
    segment-sum masks).
  - Layer 1: every core builds the full gather table G1[v] = [h(v) (512) |
    s_tgt(v) (8)] in bf16 with dense matmuls (replicated work beats an
    AllGather at this size). The edge phase gathers ALL of a node tile's
    G1[tgt] rows with ONE batched indirect DMA (ET*128 rows/instruction,
    amortizing the ~1us SWDGE fixed cost), builds 0/1 bf16 masks from
    iota-compares, and reduces messages/denominators per 128-node tile with
    bf16 PE matmuls accumulating in PSUM (segment-sum == maskT.T @ rhs).
  - Per-edge s_e = edge_attr @ a_e terms are precomputed on the host and
    streamed as tiny bf16 side inputs (kills the 16-wide edge_attr stream
    and two PE matmuls per edge tile).
  - Softmax: scores are O(+-10); exp() is computed unshifted (the
    reference's global-max shift cancels in the attention ratio; its 1e-16
    epsilon is <=1e-13 relative here). A 1e-30 epsilon guards empty nodes.
  - Layer-2 node features h2 = x@W_out are computed shard-local and
    AllGathered (bf16, ~26MB), then the same edge machinery runs with
    129-value bf16 rows. log_softmax is fused into the layer-2 epilogue.
    Each core writes its own fp32 output shard; the host concatenates.
"""

import numpy as np

import concourse.bass as bass
import concourse.bacc as bacc
import concourse.mybir as mybir
import concourse.tile as tile

F32 = mybir.dt.float32
BF16 = mybir.dt.bfloat16
I32 = mybir.dt.int32

N_CORES = 8
P = 128
H = 8            # heads
DH = 64          # hidden per head
DIN = 128        # input feature dim
DC = H * DH      # 512 concat feature dim
DOUT = 128       # layer-2 output dim
EA = 16          # edge attr dim
LRELU = 0.01
G1W = DC + H     # 520: [h | s_tgt]
G2W = DOUT + 1   # 129: [h2 | s_tgt2]
G2WX = DOUT + 2  # 130: epilogue matmul also makes s_src2
EPS0 = 1e-30


def build_program(NP, ET, repeat=1):
    """One SPMD Bass program. NP must be divisible by 8*128.

    repeat>1 re-runs the whole pipeline (for wall-clock delta timing)."""
    NT_ALL = NP // P                  # dense-phase tiles
    NSH = NP // N_CORES               # nodes per core
    NT = NSH // P                     # node tiles per core
    TS = NT * ET                      # edge tiles per core

    nc = bacc.Bacc("TRN2", target_bir_lowering=False, debug=False,
                   num_devices=N_CORES)

    # --- inputs (per-core values, identical program) ---
    XTT = nc.dram_tensor("xtt", [NT_ALL, DIN, P], BF16, kind="ExternalInput")
    XTO = nc.dram_tensor("xto", [NT, DIN, P], BF16, kind="ExternalInput")
    WC = nc.dram_tensor("wc", [DIN, DC], BF16, kind="ExternalInput")
    WA = nc.dram_tensor("wa", [DIN, 2 * H], BF16, kind="ExternalInput")
    W2X = nc.dram_tensor("w2x", [P, 4 * G2WX], BF16, kind="ExternalInput")
    IOTP = nc.dram_tensor("iotp", [P, 1], F32, kind="ExternalInput")
    IOTF = nc.dram_tensor("iotf", [P, P], BF16, kind="ExternalInput")
    IDENT = nc.dram_tensor("ident", [P, P], BF16, kind="ExternalInput")
    ONES1 = nc.dram_tensor("ones1", [1, P], BF16, kind="ExternalInput")
    GIDX = nc.dram_tensor("gidx", [P, TS], I32, kind="ExternalInput")
    SRCL = nc.dram_tensor("srcl", [P, TS], BF16, kind="ExternalInput")
    SRCLR = nc.dram_tensor("srclr", [1, TS * P], BF16, kind="ExternalInput")
    SE1 = nc.dram_tensor("se1", [NT, P, ET * H], BF16, kind="ExternalInput")
    SE2 = nc.dram_tensor("se2", [NT, P, ET], BF16, kind="ExternalInput")

    # --- internal DRAM ---
    G1 = nc.dram_tensor("g1", [NP, G1W], BF16)
    G2S = nc.dram_tensor("g2s", [NSH, G2W], BF16)
    G2F = nc.dram_tensor("g2f", [NP, G2W], BF16, addr_space="Shared")

    OUT = nc.dram_tensor("out", [NSH, DOUT], F32, kind="ExternalOutput")

    AX = mybir.AxisListType.X
    OP = mybir.AluOpType
    AF = mybir.ActivationFunctionType

    with tile.TileContext(nc) as tc, \
         tc.tile_pool(name="const", bufs=1) as cp:
        wc_sb = cp.tile([DIN, DC], BF16, tag="wc")
        nc.scalar.dma_start(out=wc_sb[:], in_=WC[:])
        wa_sb = cp.tile([DIN, 2 * H], BF16, tag="wa")
        nc.scalar.dma_start(out=wa_sb[:], in_=WA[:])
        w2x_sb = cp.tile([P, 4 * G2WX], BF16, tag="w2x")
        nc.scalar.dma_start(out=w2x_sb[:], in_=W2X[:])
        iotp_sb = cp.tile([P, 1], F32, tag="iotp")
        nc.scalar.dma_start(out=iotp_sb[:], in_=IOTP[:])
        iotf_sb = cp.tile([P, P], BF16, tag="iotf")
        nc.scalar.dma_start(out=iotf_sb[:], in_=IOTF[:])
        id_sb = cp.tile([P, P], BF16, tag="ident")
        nc.scalar.dma_start(out=id_sb[:], in_=IDENT[:])
        on1_sb = cp.tile([1, P], BF16, tag="ones1")
        nc.scalar.dma_start(out=on1_sb[:], in_=ONES1[:])
        gidx_sb = cp.tile([P, TS], I32, tag="gidx")
        nc.scalar.dma_start(out=gidx_sb[:], in_=GIDX[:])
        srcl_sb = cp.tile([P, TS], BF16, tag="srcl")
        nc.scalar.dma_start(out=srcl_sb[:], in_=SRCL[:])
        s2all_sb = cp.tile([P, NT], BF16, tag="s2all")
        neg1_sb = cp.tile([P, 1], F32, tag="neg1")
        nc.vector.memset(neg1_sb[:], -1.0)

        for _rep in range(repeat):
            if _rep:
                tc.strict_bb_all_engine_barrier()
            # ---------------- Phase D1: build G1 (all nodes, replicated) -------
            with tc.tile_pool(name="d1", bufs=3) as dp, \
                 tc.tile_pool(name="d1ps", bufs=2, space="PSUM") as dps:
                for i in range(NT_ALL):
                    xt = dp.tile([DIN, P], BF16, tag="xt")
                    nc.scalar.dma_start(out=xt[:], in_=XTT[i])
                    ph = dps.tile([P, DC], F32, tag="ph")
                    nc.tensor.matmul(ph[:], (xt[:]), (wc_sb[:]),
                                     start=True, stop=True)
                    ps = dps.tile([P, 2 * H], F32, tag="ps")
                    nc.tensor.matmul(ps[:], (xt[:]), (wa_sb[:]),
                                     start=True, stop=True)
                    g1t = dp.tile([P, G1W], BF16, tag="g1t")
                    nc.vector.tensor_copy(out=g1t[:, 0:DC], in_=ph[:])
                    nc.vector.tensor_copy(out=g1t[:, DC:G1W], in_=ps[:, 0:H])
                    nc.sync.dma_start(out=G1[i * P:(i + 1) * P, :], in_=g1t[:])

            tc.strict_bb_all_engine_barrier()

            # ---------------- Phase E1: layer-1 edge pass + epilogue -----------
            with tc.tile_pool(name="e1", bufs=2) as ep, \
                 tc.tile_pool(name="gat", bufs=2) as gp, \
                 tc.tile_pool(name="sc", bufs=3) as sp, \
                 tc.tile_pool(name="x", bufs=2) as xp, \
                 tc.tile_pool(name="psM", bufs=2, space="PSUM") as psM, \
                 tc.tile_pool(name="psD", bufs=1, space="PSUM") as psD, \
                 tc.tile_pool(name="psT", bufs=2, space="PSUM") as psT, \
                 tc.tile_pool(name="psV", bufs=1, space="PSUM") as psV, \
                 tc.tile_pool(name="psSG", bufs=2, space="PSUM") as psSG:
                # zero the gather slots once: OOB (pad) rows are skipped by
                # the DMA, so these lanes must never hold non-finite garbage.
                for _b in range(2):
                    gz = gp.tile([P, ET * G1W], BF16, tag="g")
                    nc.vector.memset(gz[:], 0.0)
                for nt in range(NT):
                    xto = ep.tile([DIN, P], BF16, tag="xto")
                    nc.scalar.dma_start(out=xto[:], in_=XTO[nt])
                    ps1 = psSG.tile([P, G2WX], F32, tag="sg")
                    nc.tensor.matmul(ps1[:, 0:2 * H], (xto[:]), (wa_sb[:]),
                                     start=True, stop=True)
                    s1sb = ep.tile([P, 2 * H], BF16, tag="s1sb")
                    nc.vector.tensor_copy(out=s1sb[:], in_=ps1[:, 0:2 * H])
                    se1t = ep.tile([P, ET * H], BF16, tag="se1t")
                    nc.scalar.dma_start(out=se1t[:], in_=SE1[nt])
                    srow = ep.tile([1, ET * P], BF16, tag="srow")
                    nc.scalar.dma_start(
                        out=srow[:], in_=SRCLR[0:1, nt * ET * P:(nt + 1) * ET * P])
                    g = gp.tile([P, ET * G1W], BF16, tag="g")
                    nc.gpsimd.indirect_dma_start(
                        out=g[:], out_offset=None, in_=G1[:],
                        in_offset=bass.IndirectOffsetOnAxis(
                            ap=gidx_sb[:, nt * ET:(nt + 1) * ET], axis=0),
                        bounds_check=NP - 1, oob_is_err=False,
                    )

                    pm = psM.tile([P, DC], F32, tag="pm")
                    pd = psD.tile([P, H], F32, tag="pd")
                    for et in range(ET):
                        c = nt * ET + et
                        gv = g[:, et * G1W:(et + 1) * G1W]
                        # masks: maskT[e,v] (edges on partitions) and mask[v,e]
                        maskT = sp.tile([P, P], BF16, tag="maskT")
                        nc.vector.tensor_tensor(
                            out=maskT[:],
                            in0=srcl_sb[:, c:c + 1].to_broadcast([P, P]),
                            in1=iotf_sb[:], op=OP.is_equal)
                        pbc = psT.tile([P, P], F32, tag="t128")
                        nc.tensor.matmul(pbc[:], on1_sb[:],
                                         srow[0:1, et * P:(et + 1) * P],
                                         start=True, stop=True)
                        mask = sp.tile([P, P], BF16, tag="mask")
                        nc.vector.tensor_tensor(
                            out=mask[:], in0=pbc[:],
                            in1=iotp_sb[:].to_broadcast([P, P]), op=OP.is_equal)
                        # score: (mask @ s_src) + s_tgt[tgt] + s_e, lrelu, exp
                        pv = psV.tile([P, H], F32, tag="pv")
                        nc.tensor.matmul(pv[:], (mask[:]), (s1sb[:, H:2 * H]),
                                         start=True, stop=True)
                        t1 = sp.tile([P, H], F32, tag="t1")
                        nc.vector.tensor_tensor(out=t1[:], in0=pv[:],
                                                in1=gv[:, DC:DC + H], op=OP.add)
                        sc = sp.tile([P, H], F32, tag="sc")
                        nc.vector.tensor_tensor(
                            out=sc[:], in0=t1[:],
                            in1=se1t[:, et * H:(et + 1) * H], op=OP.add)
                        ex = sp.tile([P, H], BF16, tag="ex")
                        lr = sp.tile([P, H], F32, tag="lr")
                        nc.vector.scalar_tensor_tensor(
                            out=lr[:], in0=sc[:], scalar=LRELU, in1=sc[:],
                            op0=OP.mult, op1=OP.max)
                        nc.scalar.activation(ex[:], lr[:], AF.Exp)
                        rhs = sp.tile([P, DC], BF16, tag="rhs")
                        nc.vector.tensor_tensor(
                            out=rhs[:].rearrange("p (h d) -> p h d", h=H),
                            in0=gv[:, 0:DC].rearrange("p (h d) -> p h d", h=H),
                            in1=ex[:].unsqueeze(2).to_broadcast([P, H, DH]),
                            op=OP.mult)
                        nc.tensor.matmul(pm[:], (maskT[:]), (rhs[:]),
                                         start=(et == 0), stop=(et == ET - 1))
                        nc.tensor.matmul(pd[:], (maskT[:]), (ex[:]),
                                         start=(et == 0), stop=(et == ET - 1))
                    # epilogue: divide, elu(elu(.)), h2 = x@W2, G2 shard row
                    den = ep.tile([P, H], F32, tag="den")
                    nc.vector.tensor_scalar_add(out=den[:], in0=pd[:], scalar1=EPS0)
                    rcp = ep.tile([P, H], F32, tag="rcp")
                    nc.vector.reciprocal(out=rcp[:], in_=den[:])
                    x0 = xp.tile([P, DC], BF16, tag="x0")
                    nc.vector.tensor_tensor(
                        out=x0[:].rearrange("p (h d) -> p h d", h=H),
                        in0=pm[:].rearrange("p (h d) -> p h d", h=H),
                        in1=rcp[:].unsqueeze(2).to_broadcast([P, H, DH]),
                        op=OP.mult)
                    xa = xp.tile([P, DC], BF16, tag="xa")
                    nc.vector.tensor_scalar_min(out=xa[:], in0=x0[:], scalar1=0.0)
                    xb = xp.tile([P, DC], BF16, tag="xb")
                    nc.scalar.activation(xb[:], xa[:], AF.Exp)
                    xd = xp.tile([P, DC], BF16, tag="xd")
                    nc.scalar.activation(xd[:], xb[:], AF.Exp, bias=neg1_sb[:])
                    x1 = xp.tile([P, DC], BF16, tag="x1")
                    nc.vector.scalar_tensor_tensor(
                        out=x1[:], in0=xd[:], scalar=-1.0, in1=x0[:],
                        op0=OP.add, op1=OP.max)
                    pg2 = psSG.tile([P, G2WX], F32, tag="sg")
                    for c4 in range(4):
                        pxT = psT.tile([P, P], F32, tag="t128")
                        nc.tensor.transpose(pxT[:], x1[:, c4 * P:(c4 + 1) * P],
                                            id_sb[:])
                        xTs = ep.tile([P, P], BF16, tag="xTs")
                        nc.vector.tensor_copy(out=xTs[:], in_=pxT[:])
                        nc.tensor.matmul(
                            pg2[:], (xTs[:]),
                            (w2x_sb[:, c4 * G2WX:(c4 + 1) * G2WX]),
                            start=(c4 == 0), stop=(c4 == 3))
                    g2t = ep.tile([P, G2W], BF16, tag="g2t")
                    nc.vector.tensor_copy(out=g2t[:], in_=pg2[:, 0:G2W])
                    nc.vector.tensor_copy(out=s2all_sb[:, nt:nt + 1],
                                          in_=pg2[:, G2W:G2WX])
                    nc.sync.dma_start(out=G2S[nt * P:(nt + 1) * P, :], in_=g2t[:])

            tc.strict_bb_all_engine_barrier()

            # ---------------- AllGather G2 shard -> full table -----------------
            with tc.tile_critical():
                with nc.semaphore() as cc_sem:
                    nc.gpsimd.collective_compute(
                        "AllGather", OP.bypass,
                        replica_groups=[list(range(N_CORES))],
                        ins=[G2S[:]], outs=[G2F[:]],
                    ).then_inc(cc_sem, 1)
                    nc.gpsimd.wait_ge(cc_sem, 1)

            tc.strict_bb_all_engine_barrier()

            # ---------------- Phase E2: layer-2 edge pass + log_softmax --------
            with tc.tile_pool(name="e2", bufs=2) as ep, \
                 tc.tile_pool(name="gat2", bufs=2) as gp, \
                 tc.tile_pool(name="sc2", bufs=3) as sp, \
                 tc.tile_pool(name="psM2", bufs=2, space="PSUM") as psM, \
                 tc.tile_pool(name="psT2", bufs=2, space="PSUM") as psT, \
                 tc.tile_pool(name="psV2", bufs=1, space="PSUM") as psV:
                for _b in range(2):
                    gz = gp.tile([P, ET * G2W], BF16, tag="g")
                    nc.vector.memset(gz[:], 0.0)
                for nt in range(NT):
                    se2t = ep.tile([P, ET], BF16, tag="se2t")
                    nc.scalar.dma_start(out=se2t[:], in_=SE2[nt])
                    srow = ep.tile([1, ET * P], BF16, tag="srow")
                    nc.scalar.dma_start(
                        out=srow[:], in_=SRCLR[0:1, nt * ET * P:(nt + 1) * ET * P])
                    g = gp.tile([P, ET * G2W], BF16, tag="g")
                    nc.gpsimd.indirect_dma_start(
                        out=g[:], out_offset=None, in_=G2F[:],
                        in_offset=bass.IndirectOffsetOnAxis(
                            ap=gidx_sb[:, nt * ET:(nt + 1) * ET], axis=0),
                        bounds_check=NP - 1, oob_is_err=False,
                    )
                    pm = psM.tile([P, DOUT + 1], F32, tag="pm")
                    for et in range(ET):
                        c = nt * ET + et
                        gv = g[:, et * G2W:(et + 1) * G2W]
                        maskT = sp.tile([P, P], BF16, tag="maskT")
                        nc.vector.tensor_tensor(
                            out=maskT[:],
                            in0=srcl_sb[:, c:c + 1].to_broadcast([P, P]),
                            in1=iotf_sb[:], op=OP.is_equal)
                        pbc = psT.tile([P, P], F32, tag="t128")
                        nc.tensor.matmul(pbc[:], on1_sb[:],
                                         srow[0:1, et * P:(et + 1) * P],
                                         start=True, stop=True)
                        mask = sp.tile([P, P], BF16, tag="mask")
                        nc.vector.tensor_tensor(
                            out=mask[:], in0=pbc[:],
                            in1=iotp_sb[:].to_broadcast([P, P]), op=OP.is_equal)
                        pv = psV.tile([P, 1], F32, tag="pv")
                        nc.tensor.matmul(pv[:], (mask[:]),
                                         (s2all_sb[:, nt:nt + 1]),
                                         start=True, stop=True)
                        t1 = sp.tile([P, 1], F32, tag="t1")
                        nc.vector.tensor_tensor(out=t1[:], in0=pv[:],
                                                in1=gv[:, DOUT:G2W], op=OP.add)
                        sc = sp.tile([P, 1], F32, tag="sc")
                        nc.vector.tensor_tensor(
                            out=sc[:], in0=t1[:],
                            in1=se2t[:, et:et + 1], op=OP.add)
                        lr = sp.tile([P, 1], F32, tag="lr")
                        nc.vector.scalar_tensor_tensor(
                            out=lr[:], in0=sc[:], scalar=LRELU, in1=sc[:],
                            op0=OP.mult, op1=OP.max)
                        ex = sp.tile([P, 1], BF16, tag="ex")
                        nc.scalar.activation(ex[:], lr[:], AF.Exp)
                        rhs = sp.tile([P, DOUT + 1], BF16, tag="rhs")
                        nc.vector.tensor_tensor(
                            out=rhs[:, 0:DOUT], in0=gv[:, 0:DOUT],
                            in1=ex[:].to_broadcast([P, DOUT]), op=OP.mult)
                        nc.vector.tensor_copy(out=rhs[:, DOUT:DOUT + 1], in_=ex[:])
                        nc.tensor.matmul(pm[:], (maskT[:]), (rhs[:]),
                                         start=(et == 0), stop=(et == ET - 1))
                    den = ep.tile([P, 1], F32, tag="den")
                    nc.vector.tensor_scalar_add(out=den[:], in0=pm[:, DOUT:DOUT + 1],
                                                scalar1=EPS0)
                    rcp = ep.tile([P, 1], F32, tag="rcp")
                    nc.vector.reciprocal(out=rcp[:], in_=den[:])
                    h2q = ep.tile([P, DOUT], F32, tag="h2q")
                    nc.vector.tensor_scalar_mul(out=h2q[:], in0=pm[:, 0:DOUT],
                                                scalar1=rcp[:])
                    # final elu(h2') as in _gat_layer
                    ha = ep.tile([P, DOUT], F32, tag="ha")
                    nc.vector.tensor_scalar_min(out=ha[:], in0=h2q[:], scalar1=0.0)
                    hb = ep.tile([P, DOUT], F32, tag="hb")
                    nc.scalar.activation(hb[:], ha[:], AF.Exp)
                    h2p = ep.tile([P, DOUT], F32, tag="h2p")
                    nc.vector.scalar_tensor_tensor(
                        out=h2p[:], in0=hb[:], scalar=-1.0, in1=h2q[:],
                        op0=OP.add, op1=OP.max)
                    rmax = ep.tile([P, 1], F32, tag="rmax")
                    nc.vector.tensor_reduce(out=rmax[:], in_=h2p[:], axis=AX,
                                            op=OP.max)
                    z = ep.tile([P, DOUT], F32, tag="z")
                    nc.vector.tensor_scalar_sub(out=z[:], in0=h2p[:],
                                                scalar1=rmax[:])
                    ez = ep.tile([P, DOUT], F32, tag="ez")
                    ssum = ep.tile([P, 1], F32, tag="ssum")
                    nc.scalar.activation(ez[:], z[:], AF.Exp, accum_out=ssum[:])
                    lnz = ep.tile([P, 1], F32, tag="lnz")
                    nc.scalar.activation(lnz[:], ssum[:], AF.Ln)
                    outt = ep.tile([P, DOUT], F32, tag="outt")
                    nc.vector.tensor_scalar_sub(out=outt[:], in0=z[:],
                                                scalar1=lnz[:])
                    nc.sync.dma_start(out=OUT[nt * P:(nt + 1) * P, :], in_=outt[:])

    nc.finalize()
    return nc


def preprocess(X, edge_index, edge_attr, W_heads, a_heads, W_out, a_out,
               NP=None):
    """Host-side index/weight preprocessing. Returns (in_maps, meta)."""
    import ml_dtypes
    N = X.shape[0]
    E = edge_index.shape[1]
    if NP is None:
        NP = ((N + N_CORES * P - 1) // (N_CORES * P)) * (N_CORES * P)
    NSH = NP // N_CORES
    NT = NSH // P
    NT_ALL = NP // P

    src = np.asarray(edge_index[0], dtype=np.int64)
    tgt = np.asarray(edge_index[1], dtype=np.int64)
    gt = src >> 7                       # global 128-node tile of src
    order = np.argsort(gt, kind="stable")
    cnt = np.bincount(gt, minlength=NP // P)
    ET = int(np.ceil(cnt.max() / P))
    TS = NT * ET

    starts = np.concatenate([[0], np.cumsum(cnt)])
    gs = gt[order]
    pos = np.arange(E) - starts[gs]
    core = gs // NT
    col = (gs % NT) * ET + pos // P
    lane = pos % P

    gidx = np.full((N_CORES, P, TS), NP, np.int32)
    gidx[core, lane, col] = tgt[order]
    srcl = np.full((N_CORES, P, TS), -1.0, ml_dtypes.bfloat16)
    srcl[core, lane, col] = (src[order] & 127).astype(ml_dtypes.bfloat16)
    srclr = np.full((N_CORES, 1, TS * P), -1.0, ml_dtypes.bfloat16)
    srclr[core, 0, col * P + lane] = (src[order] & 127).astype(
        ml_dtypes.bfloat16)

    ea = np.asarray(edge_attr, np.float32)
    ah = np.asarray(a_heads, np.float32)
    ao = np.asarray(a_out, np.float32)
    se1_e = ea @ ah[:, 2 * DH:2 * DH + EA].T            # [E, H]
    se2_e = ea @ ao[2 * DOUT:2 * DOUT + EA]             # [E]
    se1 = np.zeros((N_CORES, NT, P, ET, H), ml_dtypes.bfloat16)
    se1[core, col // ET, lane, col % ET] = se1_e[order].astype(
        ml_dtypes.bfloat16)
    se1 = se1.reshape(N_CORES, NT, P, ET * H)
    se2 = np.zeros((N_CORES, NT, P, ET), ml_dtypes.bfloat16)
    se2[core, col // ET, lane, col % ET] = se2_e[order].astype(
        ml_dtypes.bfloat16)

    Xp = np.zeros((NP, DIN), np.float32)
    Xp[:N] = np.asarray(X, np.float32)
    XTT = np.ascontiguousarray(
        Xp.reshape(NT_ALL, P, DIN).transpose(0, 2, 1)).astype(
        ml_dtypes.bfloat16)

    Wh = np.asarray(W_heads, np.float32)
    Wo = np.asarray(W_out, np.float32)
    WC = np.ascontiguousarray(
        Wh.transpose(1, 0, 2).reshape(DIN, DC)).astype(ml_dtypes.bfloat16)
    # WA columns: [s_tgt (0:H) | s_src (H:2H)] to match the G1 row layout
    WA = np.concatenate([
        np.einsum("hkj,hj->kh", Wh, ah[:, DH:2 * DH]),
        np.einsum("hkj,hj->kh", Wh, ah[:, :DH])], axis=1).astype(
        ml_dtypes.bfloat16)
    # W2X columns per 128-row chunk: [h2 (128) | s_tgt2 | s_src2]
    base = np.concatenate(
        [Wo, (Wo @ ao[DOUT:2 * DOUT])[:, None], (Wo @ ao[:DOUT])[:, None]],
        axis=1).astype(np.float32)                      # [DC, G2WX]
    W2X = np.ascontiguousarray(
        base.reshape(4, P, G2WX).transpose(1, 0, 2).reshape(P, 4 * G2WX)
    ).astype(ml_dtypes.bfloat16)
    IOTP = np.arange(P, dtype=np.float32)[:, None]
    IOTF = np.tile(np.arange(P, dtype=np.float32)[None, :], (P, 1)).astype(
        ml_dtypes.bfloat16)
    IDENT = np.eye(P, dtype=np.float32).astype(ml_dtypes.bfloat16)
    ONES1 = np.ones((1, P), ml_dtypes.bfloat16)

    in_maps = []
    for c in range(N_CORES):
        base_t = c * NT
        in_maps.append({
            "xtt": XTT,
            "xto": XTT[base_t:base_t + NT],
            "wc": WC, "wa": WA, "w2x": W2X,
            "iotp": IOTP, "iotf": IOTF, "ident": IDENT, "ones1": ONES1,
            "gidx": gidx[c], "srcl": srcl[c], "srclr": srclr[c],
            "se1": se1[c], "se2": se2[c],
        })
    meta = dict(N=N, NP=NP, ET=ET)
    return in_maps, meta


def make_runner(nc, n_cores=N_CORES):
    """Build a reusable jitted SPMD executor for a finalized Bass module.

    Returns run(in_maps, n_iters=0) -> (per-core output dicts, seconds/iter).
    n_iters=0 executes once (timing NaN); n_iters>0 times that many extra
    executions with device-resident inputs.
    """
    import time
    import jax
    from jax.sharding import Mesh, PartitionSpec
    from jax.experimental.shard_map import shard_map
    from concourse import bass2jax
    from concourse.bass2jax import _bass_exec_p, partition_id_tensor

    bass2jax.install_neuronx_cc_hook()
    partition_name = nc.partition_id_tensor.name if nc.partition_id_tensor else None
    in_names, out_names, out_avals, zero_outs = [], [], [], []
    for alloc in nc.m.functions[0].allocations:
        if not isinstance(alloc, mybir.MemoryLocationSet):
            continue
        name = alloc.memorylocations[0].name
        if alloc.kind == "ExternalInput":
            if name != partition_name:
                in_names.append(name)
        elif alloc.kind == "ExternalOutput":
            out_names.append(name)
            shape = tuple(alloc.tensor_shape)
            dtype = mybir.dt.np(alloc.dtype)
            out_avals.append(jax.core.ShapedArray(shape, dtype))
            zero_outs.append(np.zeros(shape, dtype))
    n_params = len(in_names)
    all_in_names = list(in_names) + list(out_names)
    if partition_name is not None:
        all_in_names.append(partition_name)

    def _body(*args):
        operands = list(args)
        if partition_name is not None:
            operands.append(partition_id_tensor())
        outs = _bass_exec_p.bind(
            *operands,
            out_avals=tuple(out_avals),
            in_names=tuple(all_in_names),
            out_names=tuple(out_names),
            lowering_input_output_aliases=(),
            sim_require_finite=True,
            sim_require_nnan=True,
            nc=nc,
        )
        return tuple(outs)

    devices = jax.devices()[:n_cores]
    mesh = Mesh(np.asarray(devices), ("core",))
    in_specs = (PartitionSpec("core"),) * (n_params + len(out_names))
    out_specs = (PartitionSpec("core"),) * len(out_names)
    sharded = jax.jit(
        shard_map(_body, mesh=mesh, in_specs=in_specs, out_specs=out_specs,
                  check_rep=False),
        keep_unused=True,
    )

    def run(in_maps, n_iters=0):
        per_core = [[np.asarray(m[name]) for name in in_names] for m in in_maps]
        concat_in = [
            np.concatenate([per_core[c][i] for c in range(n_cores)], axis=0)
            for i in range(n_params)
        ]
        concat_zeros = [
            np.zeros((n_cores * z.shape[0], *z.shape[1:]), z.dtype)
            for z in zero_outs
        ]
        args = [jax.device_put(a) for a in concat_in]
        args += [jax.device_put(a) for a in concat_zeros]
        out = sharded(*args)
        jax.block_until_ready(out)
        dt = float("nan")
        if n_iters:
            t0 = time.perf_counter()
            for _ in range(n_iters):
                out = sharded(*args)
                jax.block_until_ready(out)
            dt = (time.perf_counter() - t0) / n_iters
        results = [
            {
                name: np.asarray(out[i]).reshape(n_cores, *out_avals[i].shape)[c]
                for i, name in enumerate(out_names)
            }
            for c in range(n_cores)
        ]
        return results, dt

    return run


_RUNNER_CACHE = {}


def _get_runner(NP, ET, repeat=1):
    key = (NP, ET, repeat)
    if key not in _RUNNER_CACHE:
        nc = build_program(NP, ET, repeat=repeat)
        _RUNNER_CACHE[key] = make_runner(nc, N_CORES)
    return _RUNNER_CACHE[key]


def kernel(X, edge_index, edge_attr, W_heads, a_heads, W_out, a_out):
    in_maps, meta = preprocess(X, edge_index, edge_attr, W_heads, a_heads,
                               W_out, a_out)
    run = _get_runner(meta["NP"], meta["ET"])
    results, _ = run(in_maps, n_iters=0)
    out = np.concatenate([results[c]["out"] for c in range(N_CORES)], axis=0)
    return out[:meta["N"]].astype(np.float32)


# revision 46
# speedup vs baseline: 47.4830x; 47.4830x over previous
"""Trainium2 Bass kernel for a 2-layer multi-head GAT (gnn_message_passing).

Strategy (8 NeuronCores, SPMD), v3:
  - Nodes padded to NP = ceil(N/1024)*1024, split into 8 contiguous shards.
    Edges are assigned to the core that owns their SRC node, sorted/grouped
    by 128-node tile; each node tile's edge list is padded to ET edge tiles
    of 128 (static SPMD program; pad slots carry src_local=-1 and are
    SKIPPED by the gather via bounds_check, contributing zero through the
    segment-sum masks).
  - Layer 1: every core builds the full gather table G1[v] = [h(v) (512) |
    s_tgt(v) (8)] in bf16 with dense matmuls (replicated work beats an
    AllGather at this size). The edge phase gathers ALL of a node tile's
    G1[tgt] rows with ONE batched indirect DMA (ET*128 rows/instruction,
    amortizing the ~1us SWDGE fixed cost), builds 0/1 bf16 masks from
    iota-compares, and reduces messages/denominators per 128-node tile with
    bf16 PE matmuls accumulating in PSUM (segment-sum == maskT.T @ rhs).
  - All per-edge-tile elementwise work (masks, score adds, leaky-relu,
    exp) is batched into ONE wide DVE/ACT op per node tile; the [v,e]
    masks come from PE transposes of the [e,v] masks (bf16 PSUM + one
    batched copy per 4 tiles), so no src-broadcast stream is needed.
  - Per-edge s_e = edge_attr @ a_e terms are precomputed on the host and
    streamed as tiny bf16 side inputs.
  - Softmax: scores are O(+-10); exp() is computed unshifted (the
    reference's global-max shift cancels in the attention ratio; its 1e-16
    epsilon is <=1e-13 relative here). A 1e-30 epsilon guards empty nodes.
  - Layer-2 node features h2 = x@W_out are computed shard-local and
    AllGathered (bf16, ~26MB), then the same edge machinery runs with
    129-value bf16 rows. log_softmax is fused into the layer-2 epilogue,
    with the Ln over all node tiles batched at the end (one ACT table
    load). Each core writes its own fp32 output shard; the host concats.
"""

import numpy as np

import concourse.bass as bass
import concourse.bacc as bacc
import concourse.mybir as mybir
import concourse.tile as tile

F32 = mybir.dt.float32
BF16 = mybir.dt.bfloat16
I32 = mybir.dt.int32

N_CORES = 8
P = 128
H = 8            # heads
DH = 64          # hidden per head
DIN = 128        # input feature dim
DC = H * DH      # 512 concat feature dim
DOUT = 128       # layer-2 output dim
EA = 16          # edge attr dim
LRELU = 0.01
G1W = DC + H     # 520: [h | s_tgt]
G2W = DOUT + 2   # 130: [h2 | 1 | s_tgt2]
G2WX = DOUT + 2  # 130: [h2 | s_tgt2 | s_src2] from the epilogue matmul
RW2 = 144        # padded per-edge-tile stride in the E2 rhs (32B-aligned)
EPS0 = 1e-30
DB = 2           # D1 node tiles per batch
DEBUG_DUMP = False


def build_program(NP, ET, repeat=1):
    """One SPMD Bass program. NP must be divisible by 8*128.

    repeat>1 re-runs the whole pipeline (for wall-clock delta timing)."""
    NT_ALL = NP // P                  # dense-phase tiles
    NSH = NP // N_CORES               # nodes per core
    NT = NSH // P                     # node tiles per core
    TS = NT * ET                      # edge tiles per core
    assert NT_ALL % DB == 0

    NCH = 1                           # AllGather chunks (overlap with E1)
    CT = NT // NCH                    # node tiles per AG chunk
    CH = CT * P                       # rows per chunk per core

    nc = bacc.Bacc("TRN2", target_bir_lowering=False, debug=False,
                   num_devices=N_CORES)

    # --- inputs (per-core values, identical program) ---
    XTT = nc.dram_tensor("xtt", [NT_ALL, DIN, P], BF16, kind="ExternalInput")
    # per-tile stream: [xto.T (128) | se1 (ET*H)] on 128 partitions
    XSE = nc.dram_tensor("xse", [NT, P, P + ET * H], BF16,
                         kind="ExternalInput")
    WC = nc.dram_tensor("wc", [DIN, DC], BF16, kind="ExternalInput")
    WA = nc.dram_tensor("wa", [DIN, 2 * H], BF16, kind="ExternalInput")
    W2X = nc.dram_tensor("w2x", [P, 4 * G2WX], BF16, kind="ExternalInput")
    IOTF = nc.dram_tensor("iotf", [P, ET * P], BF16, kind="ExternalInput")
    IDENT = nc.dram_tensor("ident", [P, P], BF16, kind="ExternalInput")
    GIDX = nc.dram_tensor("gidx", [P, TS], I32, kind="ExternalInput")
    GIDX2 = nc.dram_tensor("gidx2", [P, TS], I32, kind="ExternalInput")
    SRCL = nc.dram_tensor("srcl", [P, TS], BF16, kind="ExternalInput")
    SE2 = nc.dram_tensor("se2", [NT, P, ET], BF16, kind="ExternalInput")

    # --- internal DRAM ---
    G1 = nc.dram_tensor("g1", [NP, G1W], BF16)
    G2S = nc.dram_tensor("g2s", [NSH, G2W], BF16)
    # chunk-major: rows ordered [chunk][rank][row-in-chunk] so each AG chunk
    # writes one contiguous region; gidx2 encodes this layout.
    G2F = nc.dram_tensor("g2f", [NP, G2W], BF16, addr_space="Shared")

    OUT = nc.dram_tensor("out", [NSH, DOUT], F32, kind="ExternalOutput")
    if DEBUG_DUMP:
        DBG1 = nc.dram_tensor("dbg1", [NSH, G1W], BF16, kind="ExternalOutput")
        DBG2 = nc.dram_tensor("dbg2", [NSH, G2W], BF16, kind="ExternalOutput")
        DBG3 = nc.dram_tensor("dbg3", [P, NT * DOUT], BF16,
                              kind="ExternalOutput")
        DBG4 = nc.dram_tensor("dbg4", [P, 3 * NT], F32, kind="ExternalOutput")
        DBG5 = nc.dram_tensor("dbg5", [P, 2 * ET + G2W + ET * P],
                              F32, kind="ExternalOutput")

    AX = mybir.AxisListType.X
    OP = mybir.AluOpType
    AF = mybir.ActivationFunctionType

    NMG = (ET + 3) // 4               # mask-transpose copy groups of 4

    with tile.TileContext(nc) as tc, \
         tc.tile_pool(name="const", bufs=1) as cp:
        wc_sb = cp.tile([DIN, DC], BF16, tag="wc")
        nc.scalar.dma_start(out=wc_sb[:], in_=WC[:])
        wa_sb = cp.tile([DIN, 2 * H], BF16, tag="wa")
        nc.scalar.dma_start(out=wa_sb[:], in_=WA[:])
        w2x_sb = cp.tile([P, 4 * G2WX], BF16, tag="w2x")
        nc.scalar.dma_start(out=w2x_sb[:], in_=W2X[:])
        iotf_sb = cp.tile([P, ET * P], BF16, tag="iotf")
        nc.scalar.dma_start(out=iotf_sb[:], in_=IOTF[:])
        id_sb = cp.tile([P, P], BF16, tag="ident")
        nc.scalar.dma_start(out=id_sb[:], in_=IDENT[:])
        gidx_sb = cp.tile([P, TS], I32, tag="gidx")
        nc.scalar.dma_start(out=gidx_sb[:], in_=GIDX[:])
        gidx2_sb = cp.tile([P, TS], I32, tag="gidx2")
        nc.scalar.dma_start(out=gidx2_sb[:], in_=GIDX2[:])
        srcl_sb = cp.tile([P, TS], BF16, tag="srcl")
        nc.scalar.dma_start(out=srcl_sb[:], in_=SRCL[:])
        s2all_sb = cp.tile([P, NT * 8], BF16, tag="s2all")
        zero8_sb = cp.tile([P, 8], BF16, tag="zero8")
        nc.vector.memset(zero8_sb[:], 0.0)
        h2p_sb = cp.tile([P, NT * DOUT], BF16, tag="h2p_all")
        stat_sb = cp.tile([P, 3 * NT], F32, tag="stat")  # [rmax | ssum | lnz]
        neg1_sb = cp.tile([P, 1], F32, tag="neg1")
        nc.vector.memset(neg1_sb[:], -1.0)

        for _rep in range(repeat):
            if _rep:
                tc.strict_bb_all_engine_barrier()
            # ---------------- Phase D1: build G1 (all nodes, replicated) -------
            with tc.tile_pool(name="d1", bufs=3) as dp, \
                 tc.tile_pool(name="d1ph", bufs=2, space="PSUM") as dph, \
                 tc.tile_pool(name="d1ps", bufs=2, space="PSUM") as dps:
                DB4 = 2 * DB
                for i in range(0, NT_ALL, DB4):
                    g1t = dp.tile([P, DB4 * G1W], BF16, tag="g1t")
                    g1w = g1t[:].rearrange("p (j w) -> p j w", j=DB4)
                    for half in range(2):
                        i2 = i + half * DB
                        xt = dp.tile([DIN, DB * P], BF16, tag="xt")
                        nc.sync.dma_start(
                            out=xt[:].rearrange("k (j p) -> k j p", j=DB),
                            in_=XTT[i2:i2 + DB].rearrange("j k p -> k j p"))
                        ph = dph.tile([P, DB * DC], F32, tag="ph")
                        ps = dps.tile([P, DB * H], F32, tag="ps")
                        for j in range(DB):
                            nc.tensor.matmul(ph[:, j * DC:(j + 1) * DC],
                                             (xt[:, j * P:(j + 1) * P]),
                                             (wc_sb[:]),
                                             start=True, stop=True)
                            nc.tensor.matmul(ps[:, j * H:(j + 1) * H],
                                             (xt[:, j * P:(j + 1) * P]),
                                             (wa_sb[:, 0:H]),
                                             start=True, stop=True)
                        hsl = g1w[:, half * DB:(half + 1) * DB, :]
                        # alternate the big PSUM->SBUF cast between DVE & ACT
                        if half == 0:
                            nc.vector.tensor_copy(
                                out=hsl[:, :, 0:DC],
                                in_=ph[:].rearrange("p (j w) -> p j w", j=DB))
                        else:
                            nc.scalar.activation(
                                hsl[:, :, 0:DC],
                                ph[:].rearrange("p (j w) -> p j w", j=DB),
                                AF.Copy)
                        nc.vector.tensor_copy(
                            out=hsl[:, :, DC:G1W],
                            in_=ps[:].rearrange("p (j w) -> p j w", j=DB))
                    nc.gpsimd.dma_start(
                        out=G1[i * P:(i + DB4) * P, :].rearrange(
                            "(j p) w -> p j w", j=DB4),
                        in_=g1t[:].rearrange("p (j w) -> p j w", j=DB4))

            tc.strict_bb_all_engine_barrier()
            if DEBUG_DUMP:
                nc.sync.dma_start(out=DBG1[:], in_=G1[0:NSH, :])
                tc.strict_bb_all_engine_barrier()

            # ---------------- Phase E1: layer-1 edge pass + epilogue -----------
            with nc.semaphore() as g2sem, \
                 nc.semaphore() as cc_sem, \
                 tc.tile_pool(name="e1", bufs=2) as ep, \
                 tc.tile_pool(name="gat", bufs=2) as gp, \
                 tc.tile_pool(name="sc", bufs=3) as sp, \
                 tc.tile_pool(name="rh", bufs=2) as rp, \
                 tc.tile_pool(name="x", bufs=2) as xp, \
                 tc.tile_pool(name="psM", bufs=2, space="PSUM") as psM, \
                 tc.tile_pool(name="psD", bufs=1, space="PSUM") as psD, \
                 tc.tile_pool(name="psT", bufs=2, space="PSUM") as psT, \
                 tc.tile_pool(name="psSG", bufs=2, space="PSUM") as psSG:
                # zero the gather slots once: OOB (pad) rows are skipped by
                # the DMA, so these lanes must never hold non-finite garbage.
                for _b in range(2):
                    gz = gp.tile([P, ET * G1W], BF16, tag="g")
                    nc.vector.memset(gz[:], 0.0)

                def e1_front(nt):
                    """Gather + masks + scores + rhs for tile nt."""
                    xse = ep.tile([P, P + ET * H], BF16, tag="xse")
                    nc.sync.dma_start(out=xse[:], in_=XSE[nt])
                    ps1 = psSG.tile([P, G2WX], F32, tag="sg")
                    nc.tensor.matmul(ps1[:, 0:2 * H], (xse[:, 0:P]),
                                     (wa_sb[:]), start=True, stop=True)
                    s1sb = ep.tile([P, 2 * H], BF16, tag="s1sb")
                    nc.vector.tensor_copy(out=s1sb[:], in_=ps1[:, 0:2 * H])
                    # NOTE: one indirect DMA per 128 rows — the multi-index
                    # offset AP path rounds index values through bf16 on HW.
                    g = gp.tile([P, ET * G1W], BF16, tag="g")
                    for et in range(ET):
                        nc.gpsimd.indirect_dma_start(
                            out=g[:, et * G1W:(et + 1) * G1W], out_offset=None,
                            in_=G1[:],
                            in_offset=bass.IndirectOffsetOnAxis(
                                ap=gidx_sb[:, nt * ET + et:nt * ET + et + 1],
                                axis=0),
                            bounds_check=NP - 1, oob_is_err=False,
                        )
                    gw = g[:].rearrange("p (et w) -> p et w", et=ET)

                    # all-edge-tile masks in one op: maskT[e, et*P+v]
                    mT = sp.tile([P, ET * P], BF16, tag="mT")
                    nc.vector.tensor_tensor(
                        out=mT[:].rearrange("p (et v) -> p et v", et=ET),
                        in0=srcl_sb[:, nt * ET:(nt + 1) * ET].unsqueeze(2)
                            .to_broadcast([P, ET, P]),
                        in1=iotf_sb[:].rearrange("p (et v) -> p et v", et=ET),
                        op=OP.is_equal)
                    # transposed masks mask[v, et*P+e] via PE, 4-at-a-time
                    mk = sp.tile([P, ET * P], BF16, tag="mk")
                    for grp in range(NMG):
                        lo = grp * 4
                        hi = min(lo + 4, ET)
                        pt = psT.tile([P, 4 * P], BF16, tag="t512")
                        for et in range(lo, hi):
                            nc.tensor.transpose(
                                pt[:, (et - lo) * P:(et - lo + 1) * P],
                                mT[:, et * P:(et + 1) * P], id_sb[:])
                        nc.scalar.activation(mk[:, lo * P:hi * P],
                                             pt[:, 0:(hi - lo) * P], AF.Copy)
                    # scores for all edge tiles: s_src via mask matmuls
                    pv = psD.tile([P, ET * H], F32, tag="pv")
                    for et in range(ET):
                        nc.tensor.matmul(pv[:, et * H:(et + 1) * H],
                                         (mk[:, et * P:(et + 1) * P]),
                                         (s1sb[:, H:2 * H]),
                                         start=True, stop=True)
                    t1 = sp.tile([P, ET * H], F32, tag="t1")
                    nc.vector.tensor_tensor(
                        out=t1[:].rearrange("p (et h) -> p et h", et=ET),
                        in0=pv[:].rearrange("p (et h) -> p et h", et=ET),
                        in1=gw[:, :, DC:G1W], op=OP.add)
                    sc = sp.tile([P, ET * H], F32, tag="sc")
                    nc.vector.tensor_tensor(out=sc[:], in0=t1[:],
                                            in1=xse[:, P:P + ET * H],
                                            op=OP.add)
                    lr = sp.tile([P, ET * H], F32, tag="lr")
                    nc.vector.scalar_tensor_tensor(
                        out=lr[:], in0=sc[:], scalar=LRELU, in1=sc[:],
                        op0=OP.mult, op1=OP.max)
                    ex = sp.tile([P, ET * H], BF16, tag="ex")
                    nc.scalar.activation(ex[:], lr[:], AF.Exp)
                    # rhs = h[tgt] * attn-numerator, all edge tiles at once
                    rhs = rp.tile([P, ET * DC], BF16, tag="rhs")
                    nc.vector.tensor_tensor(
                        out=rhs[:].rearrange("p (et h d) -> p et h d",
                                             et=ET, h=H),
                        in0=gw[:, :, 0:DC].rearrange(
                            "p et (h d) -> p et h d", h=H),
                        in1=ex[:].rearrange("p (et h) -> p et h", et=ET)
                            .unsqueeze(3).to_broadcast([P, ET, H, DH]),
                        op=OP.mult)
                    return dict(mT=mT, ex=ex, rhs=rhs)

                def e1_back(nt, st):
                    """Accumulate + epilogue for tile nt from front state."""
                    mT, ex, rhs = st["mT"], st["ex"], st["rhs"]
                    pm = psM.tile([P, DC], F32, tag="pm")
                    pd = psSG.tile([P, H], F32, tag="pd", bufs=1)
                    for et in range(ET):
                        nc.tensor.matmul(pm[:], (mT[:, et * P:(et + 1) * P]),
                                         (rhs[:, et * DC:(et + 1) * DC]),
                                         start=(et == 0), stop=(et == ET - 1))
                        nc.tensor.matmul(pd[:], (mT[:, et * P:(et + 1) * P]),
                                         (ex[:, et * H:(et + 1) * H]),
                                         start=(et == 0), stop=(et == ET - 1))
                    # epilogue: divide, elu(elu(.)), h2 = x@W2, G2 shard row
                    den = ep.tile([P, H], F32, tag="den")
                    nc.vector.tensor_scalar_add(out=den[:], in0=pd[:],
                                                scalar1=EPS0)
                    rcp = ep.tile([P, H], F32, tag="rcp")
                    nc.vector.reciprocal(out=rcp[:], in_=den[:])
                    x0 = xp.tile([P, DC], BF16, tag="x0")
                    nc.vector.tensor_tensor(
                        out=x0[:].rearrange("p (h d) -> p h d", h=H),
                        in0=pm[:].rearrange("p (h d) -> p h d", h=H),
                        in1=rcp[:].unsqueeze(2).to_broadcast([P, H, DH]),
                        op=OP.mult)
                    # xb = exp(min(x0,0)) = exp(-relu(-x0)) — both on ACT
                    xa = xp.tile([P, DC], BF16, tag="xa")
                    nc.scalar.activation(xa[:], x0[:], AF.Relu, scale=-1.0)
                    xb = xp.tile([P, DC], BF16, tag="xb")
                    nc.scalar.activation(xb[:], xa[:], AF.Exp, scale=-1.0)
                    xd = xp.tile([P, DC], BF16, tag="xd")
                    nc.scalar.activation(xd[:], xb[:], AF.Exp,
                                         bias=neg1_sb[:])
                    x1 = xp.tile([P, DC], BF16, tag="x1")
                    nc.vector.scalar_tensor_tensor(
                        out=x1[:], in0=xd[:], scalar=-1.0, in1=x0[:],
                        op0=OP.add, op1=OP.max)
                    pg2 = psSG.tile([P, G2WX], F32, tag="sg")
                    pxt = psT.tile([P, 4 * P], BF16, tag="t512")
                    for c4 in range(4):
                        nc.tensor.transpose(pxt[:, c4 * P:(c4 + 1) * P],
                                            x1[:, c4 * P:(c4 + 1) * P],
                                            id_sb[:])
                    xTs = ep.tile([P, 4 * P], BF16, tag="xTs")
                    nc.scalar.activation(xTs[:], pxt[:], AF.Copy)
                    for c4 in range(4):
                        nc.tensor.matmul(
                            pg2[:], (xTs[:, c4 * P:(c4 + 1) * P]),
                            (w2x_sb[:, c4 * G2WX:(c4 + 1) * G2WX]),
                            start=(c4 == 0), stop=(c4 == 3))
                    g2t = ep.tile([P, G2W], BF16, tag="g2t")
                    nc.scalar.activation(g2t[:, 0:DOUT], pg2[:, 0:DOUT],
                                         AF.Copy)
                    nc.vector.memset(g2t[:, DOUT:DOUT + 1], 1.0)
                    nc.vector.tensor_copy(out=g2t[:, DOUT + 1:G2W],
                                          in_=pg2[:, DOUT:DOUT + 1])
                    nc.vector.tensor_scalar_add(
                        out=s2all_sb[:, nt * 8:(nt + 1) * 8], in0=zero8_sb[:],
                        scalar1=pg2[:, G2WX - 1:G2WX])
                    wdma = nc.sync.dma_start(out=G2S[nt * P:(nt + 1) * P, :],
                                             in_=g2t[:])
                    if NCH > 1:
                        wdma.then_inc(g2sem, 16)

                def ag_chunk(c):
                    """AllGather chunk c of G2S once its tiles are written."""
                    with tc.tile_critical():
                        nc.gpsimd.wait_ge(g2sem, 16 * (c + 1) * CT)
                        nc.gpsimd.collective_compute(
                            "AllGather", OP.bypass,
                            replica_groups=[list(range(N_CORES))],
                            ins=[G2S[c * CH:(c + 1) * CH]],
                            outs=[G2F[c * N_CORES * CH:(c + 1) * N_CORES * CH]],
                        ).then_inc(cc_sem, 1)

                # software-pipelined: front(nt) runs while back(nt-1)'s
                # matmuls drain, so PE never waits on the big rhs op.
                prev = None
                for nt in range(NT):
                    st = e1_front(nt)
                    if prev is not None:
                        e1_back(nt - 1, prev)
                        if NCH > 1 and nt % CT == 0:
                            ag_chunk(nt // CT - 1)
                    prev = st
                e1_back(NT - 1, prev)
                if NCH > 1:
                    ag_chunk(NCH - 1)
                    with tc.tile_critical():
                        nc.gpsimd.wait_ge(cc_sem, NCH)

            tc.strict_bb_all_engine_barrier()
            if DEBUG_DUMP:
                nc.sync.dma_start(out=DBG2[:], in_=G2S[:])
                tc.strict_bb_all_engine_barrier()
            if NCH == 1:
                with tc.tile_critical():
                    with nc.semaphore() as ag_sem:
                        nc.gpsimd.collective_compute(
                            "AllGather", OP.bypass,
                            replica_groups=[list(range(N_CORES))],
                            ins=[G2S[:]], outs=[G2F[:]],
                        ).then_inc(ag_sem, 1)
                        nc.gpsimd.wait_ge(ag_sem, 1)
                tc.strict_bb_all_engine_barrier()

            # ---------------- Phase E2: layer-2 edge pass + log_softmax --------
            with tc.tile_pool(name="e2", bufs=2) as ep, \
                 tc.tile_pool(name="gat2", bufs=2) as gp, \
                 tc.tile_pool(name="sc2", bufs=3) as sp, \
                 tc.tile_pool(name="psM2", bufs=2, space="PSUM") as psM, \
                 tc.tile_pool(name="psT2", bufs=2, space="PSUM") as psT, \
                 tc.tile_pool(name="psV2", bufs=2, space="PSUM") as psV:
                for _b in range(2):
                    gz = gp.tile([P, ET * G2W], BF16, tag="g")
                    nc.vector.memset(gz[:], 0.0)

                def e2_front(nt):
                    se2t = ep.tile([P, ET], BF16, tag="se2t")
                    nc.sync.dma_start(out=se2t[:], in_=SE2[nt])
                    g = gp.tile([P, ET * G2W], BF16, tag="g")
                    for et in range(ET):
                        nc.gpsimd.indirect_dma_start(
                            out=g[:, et * G2W:(et + 1) * G2W], out_offset=None,
                            in_=G2F[:],
                            in_offset=bass.IndirectOffsetOnAxis(
                                ap=gidx2_sb[:, nt * ET + et:nt * ET + et + 1],
                                axis=0),
                            bounds_check=NP - 1, oob_is_err=False,
                        )
                    gw = g[:].rearrange("p (et w) -> p et w", et=ET)

                    mT = sp.tile([P, ET * P], BF16, tag="mT")
                    nc.vector.tensor_tensor(
                        out=mT[:].rearrange("p (et v) -> p et v", et=ET),
                        in0=srcl_sb[:, nt * ET:(nt + 1) * ET].unsqueeze(2)
                            .to_broadcast([P, ET, P]),
                        in1=iotf_sb[:].rearrange("p (et v) -> p et v", et=ET),
                        op=OP.is_equal)
                    mk = sp.tile([P, ET * P], BF16, tag="mk")
                    for grp in range(NMG):
                        lo = grp * 4
                        hi = min(lo + 4, ET)
                        pt = psT.tile([P, 4 * P], BF16, tag="t512")
                        for et in range(lo, hi):
                            nc.tensor.transpose(
                                pt[:, (et - lo) * P:(et - lo + 1) * P],
                                mT[:, et * P:(et + 1) * P], id_sb[:])
                        nc.scalar.activation(mk[:, lo * P:hi * P],
                                             pt[:, 0:(hi - lo) * P], AF.Copy)
                    pv = psV.tile([P, ET * 8], F32, tag="pv")
                    for et in range(ET):
                        nc.tensor.matmul(pv[:, et * 8:(et + 1) * 8],
                                         (mk[:, et * P:(et + 1) * P]),
                                         (s2all_sb[:, nt * 8:(nt + 1) * 8]),
                                         start=True, stop=True)
                    t1 = sp.tile([P, ET], F32, tag="t1")
                    nc.vector.tensor_tensor(
                        out=t1[:].unsqueeze(2),
                        in0=pv[:].rearrange("p (et j) -> p et j", et=ET)[:, :, 0:1],
                        in1=gw[:, :, G2W - 1:G2W], op=OP.add)
                    sc = sp.tile([P, ET], F32, tag="sc")
                    nc.vector.tensor_tensor(out=sc[:], in0=t1[:],
                                            in1=se2t[:], op=OP.add)
                    lr = sp.tile([P, ET], F32, tag="lr")
                    nc.vector.scalar_tensor_tensor(
                        out=lr[:], in0=sc[:], scalar=LRELU, in1=sc[:],
                        op0=OP.mult, op1=OP.max)
                    ex = sp.tile([P, ET], BF16, tag="ex")
                    nc.scalar.activation(ex[:], lr[:], AF.Exp)
                    rhs = sp.tile([P, ET * RW2], BF16, tag="rhs")
                    nc.vector.tensor_tensor(
                        out=rhs[:].rearrange("p (et w) -> p et w",
                                             et=ET)[:, :, 0:DOUT + 1],
                        in0=gw[:, :, 0:DOUT + 1],
                        in1=ex[:].unsqueeze(2).to_broadcast([P, ET, DOUT + 1]),
                        op=OP.mult)
                    if DEBUG_DUMP and nt == 0:
                        d5a = ep.tile([P, 2 * ET + ET * P], F32, tag="d5a")
                        nc.vector.tensor_copy(out=d5a[:, 0:ET], in_=sc[:])
                        nc.vector.tensor_copy(out=d5a[:, ET:2 * ET], in_=ex[:])
                        nc.vector.tensor_copy(out=d5a[:, 2 * ET:2 * ET + ET * P],
                                              in_=mk[:])
                        nc.sync.dma_start(
                            out=DBG5[:, 0:2 * ET + ET * P], in_=d5a[:])
                    return dict(mT=mT, rhs=rhs)

                def e2_back(nt, st):
                    mT, rhs = st["mT"], st["rhs"]
                    pm = psM.tile([P, DOUT + 1], F32, tag="pm")
                    for et in range(ET):
                        nc.tensor.matmul(pm[:], (mT[:, et * P:(et + 1) * P]),
                                         (rhs[:, et * RW2:et * RW2 + DOUT + 1]),
                                         start=(et == 0), stop=(et == ET - 1))
                    den = ep.tile([P, 1], F32, tag="den")
                    nc.vector.tensor_scalar_add(out=den[:],
                                                in0=pm[:, DOUT:DOUT + 1],
                                                scalar1=EPS0)
                    rcp = ep.tile([P, 1], F32, tag="rcp")
                    nc.vector.reciprocal(out=rcp[:], in_=den[:])
                    h2q = ep.tile([P, DOUT], F32, tag="h2q")
                    nc.vector.tensor_scalar_mul(out=h2q[:], in0=pm[:, 0:DOUT],
                                                scalar1=rcp[:])
                    # final elu(h2'): hb = exp(min(h2q,0)) = exp(-relu(-h2q))
                    ha = ep.tile([P, DOUT], F32, tag="ha")
                    nc.scalar.activation(ha[:], h2q[:], AF.Relu, scale=-1.0)
                    hb = ep.tile([P, DOUT], F32, tag="hb")
                    nc.scalar.activation(hb[:], ha[:], AF.Exp, scale=-1.0)
                    h2p = ep.tile([P, DOUT], F32, tag="h2p")
                    nc.vector.scalar_tensor_tensor(
                        out=h2p[:], in0=hb[:], scalar=-1.0, in1=h2q[:],
                        op0=OP.add, op1=OP.max)
                    nc.vector.tensor_reduce(out=stat_sb[:, nt:nt + 1],
                                            in_=h2p[:], axis=AX, op=OP.max)
                    rmn = ep.tile([P, 1], F32, tag="rmn")
                    nc.vector.tensor_scalar_mul(out=rmn[:],
                                                in0=stat_sb[:, nt:nt + 1],
                                                scalar1=-1.0)
                    ez = ep.tile([P, DOUT], F32, tag="ez")
                    nc.scalar.activation(
                        ez[:], h2p[:], AF.Exp, bias=rmn[:],
                        accum_out=stat_sb[:, NT + nt:NT + nt + 1])
                    nc.scalar.activation(
                        h2p_sb[:, nt * DOUT:(nt + 1) * DOUT], h2p[:],
                        AF.Copy)

                prev = None
                for nt in range(NT):
                    st = e2_front(nt)
                    if prev is not None:
                        e2_back(nt - 1, prev)
                    prev = st
                e2_back(NT - 1, prev)
                # batched log: one table load for all tiles
                nc.scalar.activation(stat_sb[:, 2 * NT:3 * NT],
                                     stat_sb[:, NT:2 * NT], AF.Ln)
                if DEBUG_DUMP:
                    nc.sync.dma_start(out=DBG3[:], in_=h2p_sb[:])
                    nc.sync.dma_start(out=DBG4[:], in_=stat_sb[:])
                with tc.tile_pool(name="fin", bufs=3) as fp:
                    for nt in range(NT):
                        outt = fp.tile([P, DOUT], F32, tag="outt")
                        nc.vector.tensor_scalar(
                            out=outt[:],
                            in0=h2p_sb[:, nt * DOUT:(nt + 1) * DOUT],
                            scalar1=stat_sb[:, nt:nt + 1],
                            scalar2=stat_sb[:, 2 * NT + nt:2 * NT + nt + 1],
                            op0=OP.subtract, op1=OP.subtract)
                        nc.sync.dma_start(out=OUT[nt * P:(nt + 1) * P, :],
                                          in_=outt[:])

    nc.finalize()
    return nc


def preprocess(X, edge_index, edge_attr, W_heads, a_heads, W_out, a_out,
               NP=None):
    """Host-side index/weight preprocessing. Returns (in_maps, meta)."""
    import ml_dtypes
    N = X.shape[0]
    E = edge_index.shape[1]
    if NP is None:
        NP = ((N + N_CORES * P - 1) // (N_CORES * P)) * (N_CORES * P)
    NSH = NP // N_CORES
    NT = NSH // P
    NT_ALL = NP // P

    src = np.asarray(edge_index[0], dtype=np.int64)
    tgt = np.asarray(edge_index[1], dtype=np.int64)
    gt = src >> 7                       # global 128-node tile of src
    order = np.argsort(gt, kind="stable")
    cnt = np.bincount(gt, minlength=NP // P)
    ET = int(np.ceil(cnt.max() / P))
    TS = NT * ET

    starts = np.concatenate([[0], np.cumsum(cnt)])
    gs = gt[order]
    pos = np.arange(E) - starts[gs]
    core = gs // NT
    col = (gs % NT) * ET + pos // P
    lane = pos % P

    gidx = np.full((N_CORES, P, TS), NP, np.int32)
    gidx[core, lane, col] = tgt[order]
    # layer-2 table layout must match build_program's NCH (1 = rank-major)
    NCH = 1
    CH = (NT // NCH) * P
    tg = tgt[order]
    tg2 = ((tg % NSH) // CH) * (N_CORES * CH) + (tg // NSH) * CH + (tg % NSH) % CH
    gidx2 = np.full((N_CORES, P, TS), NP, np.int32)
    gidx2[core, lane, col] = tg2
    srcl = np.full((N_CORES, P, TS), -1.0, ml_dtypes.bfloat16)
    srcl[core, lane, col] = (src[order] & 127).astype(ml_dtypes.bfloat16)

    ea = np.asarray(edge_attr, np.float32)
    ah = np.asarray(a_heads, np.float32)
    ao = np.asarray(a_out, np.float32)
    se1_e = ea @ ah[:, 2 * DH:2 * DH + EA].T            # [E, H]
    se2_e = ea @ ao[2 * DOUT:2 * DOUT + EA]             # [E]
    se1 = np.zeros((N_CORES, NT, P, ET, H), ml_dtypes.bfloat16)
    se1[core, col // ET, lane, col % ET] = se1_e[order].astype(
        ml_dtypes.bfloat16)
    se1 = se1.reshape(N_CORES, NT, P, ET * H)
    se2 = np.zeros((N_CORES, NT, P, ET), ml_dtypes.bfloat16)
    se2[core, col // ET, lane, col % ET] = se2_e[order].astype(
        ml_dtypes.bfloat16)

    Xp = np.zeros((NP, DIN), np.float32)
    Xp[:N] = np.asarray(X, np.float32)
    XTT = np.ascontiguousarray(
        Xp.reshape(NT_ALL, P, DIN).transpose(0, 2, 1)).astype(
        ml_dtypes.bfloat16)
    # per-core combined stream: [own xto.T | se1 tile]
    XSE = np.concatenate(
        [XTT.reshape(N_CORES, NT, P, P), se1], axis=3)      # [8, NT, P, P+ET*H]

    Wh = np.asarray(W_heads, np.float32)
    Wo = np.asarray(W_out, np.float32)
    WC = np.ascontiguousarray(
        Wh.transpose(1, 0, 2).reshape(DIN, DC)).astype(ml_dtypes.bfloat16)
    # WA columns: [s_tgt (0:H) | s_src (H:2H)] to match the G1 row layout
    WA = np.concatenate([
        np.einsum("hkj,hj->kh", Wh, ah[:, DH:2 * DH]),
        np.einsum("hkj,hj->kh", Wh, ah[:, :DH])], axis=1).astype(
        ml_dtypes.bfloat16)
    # W2X columns per 128-row chunk: [h2 (128) | s_tgt2 | s_src2]
    base = np.concatenate(
        [Wo, (Wo @ ao[DOUT:2 * DOUT])[:, None], (Wo @ ao[:DOUT])[:, None]],
        axis=1).astype(np.float32)                      # [DC, G2WX]
    W2X = np.ascontiguousarray(
        base.reshape(4, P, G2WX).transpose(1, 0, 2).reshape(P, 4 * G2WX)
    ).astype(ml_dtypes.bfloat16)
    IOTF = np.tile(np.arange(P, dtype=np.float32)[None, :],
                   (P, ET)).astype(ml_dtypes.bfloat16)
    IDENT = np.eye(P, dtype=np.float32).astype(ml_dtypes.bfloat16)

    in_maps = []
    for c in range(N_CORES):
        in_maps.append({
            "xtt": XTT,
            "xse": XSE[c],
            "wc": WC, "wa": WA, "w2x": W2X,
            "iotf": IOTF, "ident": IDENT,
            "gidx": gidx[c], "gidx2": gidx2[c], "srcl": srcl[c],
            "se2": se2[c],
        })
    meta = dict(N=N, NP=NP, ET=ET)
    return in_maps, meta


def make_runner(nc, n_cores=N_CORES):
    """Build a reusable jitted SPMD executor for a finalized Bass module.

    Returns run(in_maps, n_iters=0) -> (per-core output dicts, seconds/iter).
    n_iters=0 executes once (timing NaN); n_iters>0 times that many extra
    executions with device-resident inputs.
    """
    import time
    import jax
    from jax.sharding import Mesh, PartitionSpec
    from jax.experimental.shard_map import shard_map
    from concourse import bass2jax
    from concourse.bass2jax import _bass_exec_p, partition_id_tensor

    bass2jax.install_neuronx_cc_hook()
    partition_name = nc.partition_id_tensor.name if nc.partition_id_tensor else None
    in_names, out_names, out_avals, zero_outs = [], [], [], []
    for alloc in nc.m.functions[0].allocations:
        if not isinstance(alloc, mybir.MemoryLocationSet):
            continue
        name = alloc.memorylocations[0].name
        if alloc.kind == "ExternalInput":
            if name != partition_name:
                in_names.append(name)
        elif alloc.kind == "ExternalOutput":
            out_names.append(name)
            shape = tuple(alloc.tensor_shape)
            dtype = mybir.dt.np(alloc.dtype)
            out_avals.append(jax.core.ShapedArray(shape, dtype))
            zero_outs.append(np.zeros(shape, dtype))
    n_params = len(in_names)
    all_in_names = list(in_names) + list(out_names)
    if partition_name is not None:
        all_in_names.append(partition_name)

    def _body(*args):
        operands = list(args)
        if partition_name is not None:
            operands.append(partition_id_tensor())
        outs = _bass_exec_p.bind(
            *operands,
            out_avals=tuple(out_avals),
            in_names=tuple(all_in_names),
            out_names=tuple(out_names),
            lowering_input_output_aliases=(),
            sim_require_finite=True,
            sim_require_nnan=True,
            nc=nc,
        )
        return tuple(outs)

    devices = jax.devices()[:n_cores]
    mesh = Mesh(np.asarray(devices), ("core",))
    in_specs = (PartitionSpec("core"),) * (n_params + len(out_names))
    out_specs = (PartitionSpec("core"),) * len(out_names)
    sharded = jax.jit(
        shard_map(_body, mesh=mesh, in_specs=in_specs, out_specs=out_specs,
                  check_rep=False),
        keep_unused=True,
    )

    def run(in_maps, n_iters=0):
        per_core = [[np.asarray(m[name]) for name in in_names] for m in in_maps]
        concat_in = [
            np.concatenate([per_core[c][i] for c in range(n_cores)], axis=0)
            for i in range(n_params)
        ]
        concat_zeros = [
            np.zeros((n_cores * z.shape[0], *z.shape[1:]), z.dtype)
            for z in zero_outs
        ]
        args = [jax.device_put(a) for a in concat_in]
        args += [jax.device_put(a) for a in concat_zeros]
        out = sharded(*args)
        jax.block_until_ready(out)
        dt = float("nan")
        if n_iters:
            t0 = time.perf_counter()
            for _ in range(n_iters):
                out = sharded(*args)
                jax.block_until_ready(out)
            dt = (time.perf_counter() - t0) / n_iters
        results = [
            {
                name: np.asarray(out[i]).reshape(n_cores, *out_avals[i].shape)[c]
                for i, name in enumerate(out_names)
            }
            for c in range(n_cores)
        ]
        return results, dt

    return run


_RUNNER_CACHE = {}


def _get_runner(NP, ET, repeat=1):
    key = (NP, ET, repeat)
    if key not in _RUNNER_CACHE:
        nc = build_program(NP, ET, repeat=repeat)
        _RUNNER_CACHE[key] = make_runner(nc, N_CORES)
    return _RUNNER_CACHE[key]


def kernel(X, edge_index, edge_attr, W_heads, a_heads, W_out, a_out):
    in_maps, meta = preprocess(X, edge_index, edge_attr, W_heads, a_heads,
                               W_out, a_out)
    run = _get_runner(meta["NP"], meta["ET"])
    results, _ = run(in_maps, n_iters=0)
    out = np.concatenate([results[c]["out"] for c in range(N_CORES)], axis=0)
    return out[:meta["N"]].astype(np.float32)


# revision 47
# speedup vs baseline: 811.6684x; 17.0939x over previous
"""Trainium2 Bass kernel for a 2-layer multi-head GAT (gnn_message_passing).

Strategy (8 NeuronCores, SPMD), v3:
  - Nodes padded to NP = ceil(N/1024)*1024, split into 8 contiguous shards.
    Edges are assigned to the core that owns their SRC node, sorted/grouped
    by 128-node tile; each node tile's edge list is padded to ET edge tiles
    of 128 (static SPMD program; pad slots carry src_local=-1 and are
    SKIPPED by the gather via bounds_check, contributing zero through the
    segment-sum masks).
  - Layer 1: every core builds the full gather table G1[v] = [h(v) (512) |
    s_tgt(v) (8)] in bf16 with dense matmuls (replicated work beats an
    AllGather at this size). The edge phase gathers ALL of a node tile's
    G1[tgt] rows with ONE batched indirect DMA (ET*128 rows/instruction,
    amortizing the ~1us SWDGE fixed cost), builds 0/1 bf16 masks from
    iota-compares, and reduces messages/denominators per 128-node tile with
    bf16 PE matmuls accumulating in PSUM (segment-sum == maskT.T @ rhs).
  - All per-edge-tile elementwise work (masks, score adds, leaky-relu,
    exp) is batched into ONE wide DVE/ACT op per node tile; the [v,e]
    masks come from PE transposes of the [e,v] masks (bf16 PSUM + one
    batched copy per 4 tiles), so no src-broadcast stream is needed.
  - Per-edge s_e = edge_attr @ a_e terms are precomputed on the host and
    streamed as tiny bf16 side inputs.
  - Softmax: scores are O(+-10); exp() is computed unshifted (the
    reference's global-max shift cancels in the attention ratio; its 1e-16
    epsilon is <=1e-13 relative here). A 1e-30 epsilon guards empty nodes.
  - Layer-2 node features h2 = x@W_out are computed shard-local and
    AllGathered (bf16, ~26MB), then the same edge machinery runs with
    129-value bf16 rows. log_softmax is fused into the layer-2 epilogue,
    with the Ln over all node tiles batched at the end (one ACT table
    load). Each core writes its own fp32 output shard; the host concats.
"""

import numpy as np

import concourse.bass as bass
import concourse.bacc as bacc
import concourse.mybir as mybir
import concourse.tile as tile

F32 = mybir.dt.float32
BF16 = mybir.dt.bfloat16
I32 = mybir.dt.int32

N_CORES = 8
P = 128
H = 8            # heads
DH = 64          # hidden per head
DIN = 128        # input feature dim
DC = H * DH      # 512 concat feature dim
DOUT = 128       # layer-2 output dim
EA = 16          # edge attr dim
LRELU = 0.01
G1W = DC + H     # 520: [h | s_tgt]
G2W = DOUT + 2   # 130: [h2 | 1 | s_tgt2]
G2WX = DOUT + 2  # 130: [h2 | s_tgt2 | s_src2] from the epilogue matmul
RW2 = 144        # padded per-edge-tile stride in the E2 rhs (32B-aligned)
EPS0 = 1e-30
DB = 2           # D1 node tiles per batch
DEBUG_DUMP = False


def build_program(NP, ET, repeat=1):
    """One SPMD Bass program. NP must be divisible by 8*128.

    repeat>1 re-runs the whole pipeline (for wall-clock delta timing)."""
    NT_ALL = NP // P                  # dense-phase tiles
    NSH = NP // N_CORES               # nodes per core
    NT = NSH // P                     # node tiles per core
    TS = NT * ET                      # edge tiles per core
    assert NT_ALL % DB == 0

    NCH = 1                           # AllGather chunks (overlap with E1)
    CT = NT // NCH                    # node tiles per AG chunk
    CH = CT * P                       # rows per chunk per core

    nc = bacc.Bacc("TRN2", target_bir_lowering=False, debug=False,
                   num_devices=N_CORES)

    # --- inputs (per-core values, identical program) ---
    XTT = nc.dram_tensor("xtt", [NT_ALL, DIN, P], BF16, kind="ExternalInput")
    # per-tile stream: [xto.T (128) | se1 (ET*H)] on 128 partitions
    XSE = nc.dram_tensor("xse", [NT, P, P + ET * H], BF16,
                         kind="ExternalInput")
    WC = nc.dram_tensor("wc", [DIN, DC], BF16, kind="ExternalInput")
    WA = nc.dram_tensor("wa", [DIN, 2 * H], BF16, kind="ExternalInput")
    W2X = nc.dram_tensor("w2x", [P, 4 * G2WX], BF16, kind="ExternalInput")
    IOTF = nc.dram_tensor("iotf", [P, ET * P], BF16, kind="ExternalInput")
    IDENT = nc.dram_tensor("ident", [P, P], BF16, kind="ExternalInput")
    GIDX = nc.dram_tensor("gidx", [P, TS], I32, kind="ExternalInput")
    GIDX2 = nc.dram_tensor("gidx2", [P, TS], I32, kind="ExternalInput")
    SRCL = nc.dram_tensor("srcl", [P, TS], BF16, kind="ExternalInput")
    SE2 = nc.dram_tensor("se2", [NT, P, ET], BF16, kind="ExternalInput")

    # --- internal DRAM ---
    G1 = nc.dram_tensor("g1", [NP, G1W], BF16)
    G2S = nc.dram_tensor("g2s", [NSH, G2W], BF16)
    # chunk-major: rows ordered [chunk][rank][row-in-chunk] so each AG chunk
    # writes one contiguous region; gidx2 encodes this layout.
    G2F = nc.dram_tensor("g2f", [NP, G2W], BF16, addr_space="Shared")

    OUT = nc.dram_tensor("out", [NSH, DOUT], F32, kind="ExternalOutput")
    if DEBUG_DUMP:
        DBG1 = nc.dram_tensor("dbg1", [NSH, G1W], BF16, kind="ExternalOutput")
        DBG2 = nc.dram_tensor("dbg2", [NSH, G2W], BF16, kind="ExternalOutput")
        DBG3 = nc.dram_tensor("dbg3", [P, NT * DOUT], BF16,
                              kind="ExternalOutput")
        DBG4 = nc.dram_tensor("dbg4", [P, 3 * NT], F32, kind="ExternalOutput")
        DBG5 = nc.dram_tensor("dbg5", [P, 2 * ET + G2W + ET * P],
                              F32, kind="ExternalOutput")

    AX = mybir.AxisListType.X
    OP = mybir.AluOpType
    AF = mybir.ActivationFunctionType

    NMG = (ET + 3) // 4               # mask-transpose copy groups of 4

    with tile.TileContext(nc) as tc, \
         tc.tile_pool(name="const", bufs=1) as cp:
        wc_sb = cp.tile([DIN, DC], BF16, tag="wc")
        nc.scalar.dma_start(out=wc_sb[:], in_=WC[:])
        wa_sb = cp.tile([DIN, 2 * H], BF16, tag="wa")
        nc.scalar.dma_start(out=wa_sb[:], in_=WA[:])
        w2x_sb = cp.tile([P, 4 * G2WX], BF16, tag="w2x")
        nc.scalar.dma_start(out=w2x_sb[:], in_=W2X[:])
        iotf_sb = cp.tile([P, ET * P], BF16, tag="iotf")
        nc.scalar.dma_start(out=iotf_sb[:], in_=IOTF[:])
        id_sb = cp.tile([P, P], BF16, tag="ident")
        nc.scalar.dma_start(out=id_sb[:], in_=IDENT[:])
        gidx_sb = cp.tile([P, TS], I32, tag="gidx")
        nc.scalar.dma_start(out=gidx_sb[:], in_=GIDX[:])
        gidx2_sb = cp.tile([P, TS], I32, tag="gidx2")
        nc.scalar.dma_start(out=gidx2_sb[:], in_=GIDX2[:])
        srcl_sb = cp.tile([P, TS], BF16, tag="srcl")
        nc.scalar.dma_start(out=srcl_sb[:], in_=SRCL[:])
        s2all_sb = cp.tile([P, NT * 8], BF16, tag="s2all")
        zero8_sb = cp.tile([P, 8], BF16, tag="zero8")
        nc.vector.memset(zero8_sb[:], 0.0)
        h2p_sb = cp.tile([P, NT * DOUT], BF16, tag="h2p_all")
        stat_sb = cp.tile([P, 3 * NT], F32, tag="stat")  # [rmax | ssum | lnz]
        neg1_sb = cp.tile([P, 1], F32, tag="neg1")
        nc.vector.memset(neg1_sb[:], -1.0)

        for _rep in range(repeat):
            if _rep:
                tc.strict_bb_all_engine_barrier()
            # ---------------- Phase D1: build G1 (all nodes, replicated) -------
            with tc.tile_pool(name="d1", bufs=3) as dp, \
                 tc.tile_pool(name="d1ph", bufs=2, space="PSUM") as dph, \
                 tc.tile_pool(name="d1ps", bufs=2, space="PSUM") as dps:
                DB4 = 2 * DB
                for i in range(0, NT_ALL, DB4):
                    g1t = dp.tile([P, DB4 * G1W], BF16, tag="g1t")
                    g1w = g1t[:].rearrange("p (j w) -> p j w", j=DB4)
                    for half in range(2):
                        i2 = i + half * DB
                        xt = dp.tile([DIN, DB * P], BF16, tag="xt")
                        nc.sync.dma_start(
                            out=xt[:].rearrange("k (j p) -> k j p", j=DB),
                            in_=XTT[i2:i2 + DB].rearrange("j k p -> k j p"))
                        ph = dph.tile([P, DB * DC], F32, tag="ph")
                        ps = dps.tile([P, DB * H], F32, tag="ps")
                        for j in range(DB):
                            nc.tensor.matmul(ph[:, j * DC:(j + 1) * DC],
                                             (xt[:, j * P:(j + 1) * P]),
                                             (wc_sb[:]),
                                             start=True, stop=True)
                            nc.tensor.matmul(ps[:, j * H:(j + 1) * H],
                                             (xt[:, j * P:(j + 1) * P]),
                                             (wa_sb[:, 0:H]),
                                             start=True, stop=True)
                        hsl = g1w[:, half * DB:(half + 1) * DB, :]
                        # alternate the big PSUM->SBUF cast between DVE & ACT
                        if half == 0:
                            nc.vector.tensor_copy(
                                out=hsl[:, :, 0:DC],
                                in_=ph[:].rearrange("p (j w) -> p j w", j=DB))
                        else:
                            nc.scalar.activation(
                                hsl[:, :, 0:DC],
                                ph[:].rearrange("p (j w) -> p j w", j=DB),
                                AF.Copy)
                        nc.vector.tensor_copy(
                            out=hsl[:, :, DC:G1W],
                            in_=ps[:].rearrange("p (j w) -> p j w", j=DB))
                    nc.gpsimd.dma_start(
                        out=G1[i * P:(i + DB4) * P, :].rearrange(
                            "(j p) w -> p j w", j=DB4),
                        in_=g1t[:].rearrange("p (j w) -> p j w", j=DB4))

            tc.strict_bb_all_engine_barrier()
            if DEBUG_DUMP:
                nc.sync.dma_start(out=DBG1[:], in_=G1[0:NSH, :])
                tc.strict_bb_all_engine_barrier()

            # ---------------- Phase E1: layer-1 edge pass + epilogue -----------
            with nc.semaphore() as g2sem, \
                 nc.semaphore() as cc_sem, \
                 tc.tile_pool(name="e1", bufs=2) as ep, \
                 tc.tile_pool(name="gat", bufs=3) as gp, \
                 tc.tile_pool(name="sc", bufs=3) as sp, \
                 tc.tile_pool(name="rh", bufs=2) as rp, \
                 tc.tile_pool(name="x", bufs=2) as xp, \
                 tc.tile_pool(name="psM", bufs=2, space="PSUM") as psM, \
                 tc.tile_pool(name="psD", bufs=1, space="PSUM") as psD, \
                 tc.tile_pool(name="psT", bufs=2, space="PSUM") as psT, \
                 tc.tile_pool(name="psSG", bufs=2, space="PSUM") as psSG:
                # zero the gather slots once: OOB (pad) rows are skipped by
                # the DMA, so these lanes must never hold non-finite garbage.
                for _b in range(3):
                    gz = gp.tile([P, ET * G1W], BF16, tag="g")
                    nc.vector.memset(gz[:], 0.0)

                def e1_front(nt):
                    """Gather + masks + scores + rhs for tile nt."""
                    xse = ep.tile([P, P + ET * H], BF16, tag="xse")
                    nc.sync.dma_start(out=xse[:], in_=XSE[nt])
                    ps1 = psSG.tile([P, G2WX], F32, tag="sg")
                    nc.tensor.matmul(ps1[:, 0:2 * H], (xse[:, 0:P]),
                                     (wa_sb[:]), start=True, stop=True)
                    s1sb = ep.tile([P, 2 * H], BF16, tag="s1sb")
                    nc.vector.tensor_copy(out=s1sb[:], in_=ps1[:, 0:2 * H])
                    # NOTE: one indirect DMA per 128 rows — the multi-index
                    # offset AP path rounds index values through bf16 on HW.
                    g = gp.tile([P, ET * G1W], BF16, tag="g")
                    for et in range(ET):
                        nc.gpsimd.indirect_dma_start(
                            out=g[:, et * G1W:(et + 1) * G1W], out_offset=None,
                            in_=G1[:],
                            in_offset=bass.IndirectOffsetOnAxis(
                                ap=gidx_sb[:, nt * ET + et:nt * ET + et + 1],
                                axis=0),
                            bounds_check=NP - 1, oob_is_err=False,
                        )
                    gw = g[:].rearrange("p (et w) -> p et w", et=ET)

                    # all-edge-tile masks in one op: maskT[e, et*P+v]
                    mT = sp.tile([P, ET * P], BF16, tag="mT")
                    nc.vector.tensor_tensor(
                        out=mT[:].rearrange("p (et v) -> p et v", et=ET),
                        in0=srcl_sb[:, nt * ET:(nt + 1) * ET].unsqueeze(2)
                            .to_broadcast([P, ET, P]),
                        in1=iotf_sb[:].rearrange("p (et v) -> p et v", et=ET),
                        op=OP.is_equal)
                    # transposed masks mask[v, et*P+e] via PE, 4-at-a-time
                    mk = sp.tile([P, ET * P], BF16, tag="mk")
                    for grp in range(NMG):
                        lo = grp * 4
                        hi = min(lo + 4, ET)
                        pt = psT.tile([P, 4 * P], BF16, tag="t512")
                        for et in range(lo, hi):
                            nc.tensor.transpose(
                                pt[:, (et - lo) * P:(et - lo + 1) * P],
                                mT[:, et * P:(et + 1) * P], id_sb[:])
                        nc.scalar.activation(mk[:, lo * P:hi * P],
                                             pt[:, 0:(hi - lo) * P], AF.Copy)
                    # scores for all edge tiles: s_src via mask matmuls
                    pv = psD.tile([P, ET * H], F32, tag="pv")
                    for et in range(ET):
                        nc.tensor.matmul(pv[:, et * H:(et + 1) * H],
                                         (mk[:, et * P:(et + 1) * P]),
                                         (s1sb[:, H:2 * H]),
                                         start=True, stop=True)
                    t1 = sp.tile([P, ET * H], F32, tag="t1")
                    nc.vector.tensor_tensor(
                        out=t1[:].rearrange("p (et h) -> p et h", et=ET),
                        in0=pv[:].rearrange("p (et h) -> p et h", et=ET),
                        in1=gw[:, :, DC:G1W], op=OP.add)
                    sc = sp.tile([P, ET * H], F32, tag="sc")
                    nc.vector.tensor_tensor(out=sc[:], in0=t1[:],
                                            in1=xse[:, P:P + ET * H],
                                            op=OP.add)
                    lr = sp.tile([P, ET * H], F32, tag="lr")
                    nc.vector.scalar_tensor_tensor(
                        out=lr[:], in0=sc[:], scalar=LRELU, in1=sc[:],
                        op0=OP.mult, op1=OP.max)
                    ex = sp.tile([P, ET * H], BF16, tag="ex")
                    nc.scalar.activation(ex[:], lr[:], AF.Exp)
                    # rhs = h[tgt] * attn-numerator, all edge tiles at once
                    rhs = rp.tile([P, ET * DC], BF16, tag="rhs")
                    nc.vector.tensor_tensor(
                        out=rhs[:].rearrange("p (et h d) -> p et h d",
                                             et=ET, h=H),
                        in0=gw[:, :, 0:DC].rearrange(
                            "p et (h d) -> p et h d", h=H),
                        in1=ex[:].rearrange("p (et h) -> p et h", et=ET)
                            .unsqueeze(3).to_broadcast([P, ET, H, DH]),
                        op=OP.mult)
                    return dict(mT=mT, ex=ex, rhs=rhs)

                def e1_back(nt, st):
                    """Accumulate + epilogue for tile nt from front state."""
                    mT, ex, rhs = st["mT"], st["ex"], st["rhs"]
                    pm = psM.tile([P, DC], F32, tag="pm")
                    pd = psSG.tile([P, H], F32, tag="pd", bufs=1)
                    for et in range(ET):
                        nc.tensor.matmul(pm[:], (mT[:, et * P:(et + 1) * P]),
                                         (rhs[:, et * DC:(et + 1) * DC]),
                                         start=(et == 0), stop=(et == ET - 1))
                        nc.tensor.matmul(pd[:], (mT[:, et * P:(et + 1) * P]),
                                         (ex[:, et * H:(et + 1) * H]),
                                         start=(et == 0), stop=(et == ET - 1))
                    # epilogue: divide, elu(elu(.)), h2 = x@W2, G2 shard row
                    den = ep.tile([P, H], F32, tag="den")
                    nc.vector.tensor_scalar_add(out=den[:], in0=pd[:],
                                                scalar1=EPS0)
                    rcp = ep.tile([P, H], F32, tag="rcp")
                    nc.vector.reciprocal(out=rcp[:], in_=den[:])
                    x0 = xp.tile([P, DC], BF16, tag="x0")
                    nc.vector.tensor_tensor(
                        out=x0[:].rearrange("p (h d) -> p h d", h=H),
                        in0=pm[:].rearrange("p (h d) -> p h d", h=H),
                        in1=rcp[:].unsqueeze(2).to_broadcast([P, H, DH]),
                        op=OP.mult)
                    # xb = exp(min(x0,0)) = exp(-relu(-x0)) — both on ACT
                    xa = xp.tile([P, DC], BF16, tag="xa")
                    nc.scalar.activation(xa[:], x0[:], AF.Relu, scale=-1.0)
                    xb = xp.tile([P, DC], BF16, tag="xb")
                    nc.scalar.activation(xb[:], xa[:], AF.Exp, scale=-1.0)
                    xd = xp.tile([P, DC], BF16, tag="xd")
                    nc.scalar.activation(xd[:], xb[:], AF.Exp,
                                         bias=neg1_sb[:])
                    x1 = xp.tile([P, DC], BF16, tag="x1")
                    nc.vector.scalar_tensor_tensor(
                        out=x1[:], in0=xd[:], scalar=-1.0, in1=x0[:],
                        op0=OP.add, op1=OP.max)
                    pg2 = psSG.tile([P, G2WX], F32, tag="sg")
                    pxt = psT.tile([P, 4 * P], BF16, tag="t512")
                    for c4 in range(4):
                        nc.tensor.transpose(pxt[:, c4 * P:(c4 + 1) * P],
                                            x1[:, c4 * P:(c4 + 1) * P],
                                            id_sb[:])
                    xTs = ep.tile([P, 4 * P], BF16, tag="xTs")
                    nc.scalar.activation(xTs[:], pxt[:], AF.Copy)
                    for c4 in range(4):
                        nc.tensor.matmul(
                            pg2[:], (xTs[:, c4 * P:(c4 + 1) * P]),
                            (w2x_sb[:, c4 * G2WX:(c4 + 1) * G2WX]),
                            start=(c4 == 0), stop=(c4 == 3))
                    g2t = ep.tile([P, G2W], BF16, tag="g2t")
                    nc.scalar.activation(g2t[:, 0:DOUT], pg2[:, 0:DOUT],
                                         AF.Copy)
                    nc.vector.memset(g2t[:, DOUT:DOUT + 1], 1.0)
                    nc.vector.tensor_copy(out=g2t[:, DOUT + 1:G2W],
                                          in_=pg2[:, DOUT:DOUT + 1])
                    nc.vector.tensor_scalar_add(
                        out=s2all_sb[:, nt * 8:(nt + 1) * 8], in0=zero8_sb[:],
                        scalar1=pg2[:, G2WX - 1:G2WX])
                    wdma = nc.sync.dma_start(out=G2S[nt * P:(nt + 1) * P, :],
                                             in_=g2t[:])
                    if NCH > 1:
                        wdma.then_inc(g2sem, 16)

                def ag_chunk(c):
                    """AllGather chunk c of G2S once its tiles are written."""
                    with tc.tile_critical():
                        nc.gpsimd.wait_ge(g2sem, 16 * (c + 1) * CT)
                        nc.gpsimd.collective_compute(
                            "AllGather", OP.bypass,
                            replica_groups=[list(range(N_CORES))],
                            ins=[G2S[c * CH:(c + 1) * CH]],
                            outs=[G2F[c * N_CORES * CH:(c + 1) * N_CORES * CH]],
                        ).then_inc(cc_sem, 1)

                # software-pipelined: front(nt) runs while back(nt-1)'s
                # matmuls drain, so PE never waits on the big rhs op.
                prev = None
                for nt in range(NT):
                    st = e1_front(nt)
                    if prev is not None:
                        e1_back(nt - 1, prev)
                        if NCH > 1 and nt % CT == 0:
                            ag_chunk(nt // CT - 1)
                    prev = st
                e1_back(NT - 1, prev)
                if NCH > 1:
                    ag_chunk(NCH - 1)
                    with tc.tile_critical():
                        nc.gpsimd.wait_ge(cc_sem, NCH)

            tc.strict_bb_all_engine_barrier()
            if DEBUG_DUMP:
                nc.sync.dma_start(out=DBG2[:], in_=G2S[:])
                tc.strict_bb_all_engine_barrier()
            if NCH == 1:
                with tc.tile_critical():
                    with nc.semaphore() as ag_sem:
                        nc.gpsimd.collective_compute(
                            "AllGather", OP.bypass,
                            replica_groups=[list(range(N_CORES))],
                            ins=[G2S[:]], outs=[G2F[:]],
                        ).then_inc(ag_sem, 1)
                        nc.gpsimd.wait_ge(ag_sem, 1)
                tc.strict_bb_all_engine_barrier()

            # ---------------- Phase E2: layer-2 edge pass + log_softmax --------
            with tc.tile_pool(name="e2", bufs=2) as ep, \
                 tc.tile_pool(name="gat2", bufs=2) as gp, \
                 tc.tile_pool(name="sc2", bufs=3) as sp, \
                 tc.tile_pool(name="psM2", bufs=2, space="PSUM") as psM, \
                 tc.tile_pool(name="psT2", bufs=2, space="PSUM") as psT, \
                 tc.tile_pool(name="psV2", bufs=2, space="PSUM") as psV:
                for _b in range(2):
                    gz = gp.tile([P, ET * G2W], BF16, tag="g")
                    nc.vector.memset(gz[:], 0.0)

                def e2_front(nt):
                    se2t = ep.tile([P, ET], BF16, tag="se2t")
                    nc.sync.dma_start(out=se2t[:], in_=SE2[nt])
                    g = gp.tile([P, ET * G2W], BF16, tag="g")
                    for et in range(ET):
                        nc.gpsimd.indirect_dma_start(
                            out=g[:, et * G2W:(et + 1) * G2W], out_offset=None,
                            in_=G2F[:],
                            in_offset=bass.IndirectOffsetOnAxis(
                                ap=gidx2_sb[:, nt * ET + et:nt * ET + et + 1],
                                axis=0),
                            bounds_check=NP - 1, oob_is_err=False,
                        )
                    gw = g[:].rearrange("p (et w) -> p et w", et=ET)

                    mT = sp.tile([P, ET * P], BF16, tag="mT")
                    nc.vector.tensor_tensor(
                        out=mT[:].rearrange("p (et v) -> p et v", et=ET),
                        in0=srcl_sb[:, nt * ET:(nt + 1) * ET].unsqueeze(2)
                            .to_broadcast([P, ET, P]),
                        in1=iotf_sb[:].rearrange("p (et v) -> p et v", et=ET),
                        op=OP.is_equal)
                    mk = sp.tile([P, ET * P], BF16, tag="mk")
                    for grp in range(NMG):
                        lo = grp * 4
                        hi = min(lo + 4, ET)
                        pt = psT.tile([P, 4 * P], BF16, tag="t512")
                        for et in range(lo, hi):
                            nc.tensor.transpose(
                                pt[:, (et - lo) * P:(et - lo + 1) * P],
                                mT[:, et * P:(et + 1) * P], id_sb[:])
                        nc.scalar.activation(mk[:, lo * P:hi * P],
                                             pt[:, 0:(hi - lo) * P], AF.Copy)
                    pv = psV.tile([P, ET * 8], F32, tag="pv")
                    for et in range(ET):
                        nc.tensor.matmul(pv[:, et * 8:(et + 1) * 8],
                                         (mk[:, et * P:(et + 1) * P]),
                                         (s2all_sb[:, nt * 8:(nt + 1) * 8]),
                                         start=True, stop=True)
                    t1 = sp.tile([P, ET], F32, tag="t1")
                    nc.vector.tensor_tensor(
                        out=t1[:].unsqueeze(2),
                        in0=pv[:].rearrange("p (et j) -> p et j", et=ET)[:, :, 0:1],
                        in1=gw[:, :, G2W - 1:G2W], op=OP.add)
                    sc = sp.tile([P, ET], F32, tag="sc")
                    nc.vector.tensor_tensor(out=sc[:], in0=t1[:],
                                            in1=se2t[:], op=OP.add)
                    lr = sp.tile([P, ET], F32, tag="lr")
                    nc.vector.scalar_tensor_tensor(
                        out=lr[:], in0=sc[:], scalar=LRELU, in1=sc[:],
                        op0=OP.mult, op1=OP.max)
                    ex = sp.tile([P, ET], BF16, tag="ex")
                    nc.scalar.activation(ex[:], lr[:], AF.Exp)
                    rhs = sp.tile([P, ET * RW2], BF16, tag="rhs")
                    nc.vector.tensor_tensor(
                        out=rhs[:].rearrange("p (et w) -> p et w",
                                             et=ET)[:, :, 0:DOUT + 1],
                        in0=gw[:, :, 0:DOUT + 1],
                        in1=ex[:].unsqueeze(2).to_broadcast([P, ET, DOUT + 1]),
                        op=OP.mult)
                    if DEBUG_DUMP and nt == 0:
                        d5a = ep.tile([P, 2 * ET + ET * P], F32, tag="d5a")
                        nc.vector.tensor_copy(out=d5a[:, 0:ET], in_=sc[:])
                        nc.vector.tensor_copy(out=d5a[:, ET:2 * ET], in_=ex[:])
                        nc.vector.tensor_copy(out=d5a[:, 2 * ET:2 * ET + ET * P],
                                              in_=mk[:])
                        nc.sync.dma_start(
                            out=DBG5[:, 0:2 * ET + ET * P], in_=d5a[:])
                    return dict(mT=mT, rhs=rhs)

                def e2_back(nt, st):
                    mT, rhs = st["mT"], st["rhs"]
                    pm = psM.tile([P, DOUT + 1], F32, tag="pm")
                    for et in range(ET):
                        nc.tensor.matmul(pm[:], (mT[:, et * P:(et + 1) * P]),
                                         (rhs[:, et * RW2:et * RW2 + DOUT + 1]),
                                         start=(et == 0), stop=(et == ET - 1))
                    den = ep.tile([P, 1], F32, tag="den")
                    nc.vector.tensor_scalar_add(out=den[:],
                                                in0=pm[:, DOUT:DOUT + 1],
                                                scalar1=EPS0)
                    rcp = ep.tile([P, 1], F32, tag="rcp")
                    nc.vector.reciprocal(out=rcp[:], in_=den[:])
                    h2q = ep.tile([P, DOUT], F32, tag="h2q")
                    nc.vector.tensor_scalar_mul(out=h2q[:], in0=pm[:, 0:DOUT],
                                                scalar1=rcp[:])
                    # final elu(h2'): hb = exp(min(h2q,0)) = exp(-relu(-h2q))
                    ha = ep.tile([P, DOUT], F32, tag="ha")
                    nc.scalar.activation(ha[:], h2q[:], AF.Relu, scale=-1.0)
                    hb = ep.tile([P, DOUT], F32, tag="hb")
                    nc.scalar.activation(hb[:], ha[:], AF.Exp, scale=-1.0)
                    h2p = ep.tile([P, DOUT], F32, tag="h2p")
                    nc.vector.scalar_tensor_tensor(
                        out=h2p[:], in0=hb[:], scalar=-1.0, in1=h2q[:],
                        op0=OP.add, op1=OP.max)
                    nc.vector.tensor_reduce(out=stat_sb[:, nt:nt + 1],
                                            in_=h2p[:], axis=AX, op=OP.max)
                    rmn = ep.tile([P, 1], F32, tag="rmn")
                    nc.vector.tensor_scalar_mul(out=rmn[:],
                                                in0=stat_sb[:, nt:nt + 1],
                                                scalar1=-1.0)
                    ez = ep.tile([P, DOUT], F32, tag="ez")
                    nc.scalar.activation(
                        ez[:], h2p[:], AF.Exp, bias=rmn[:],
                        accum_out=stat_sb[:, NT + nt:NT + nt + 1])
                    nc.scalar.activation(
                        h2p_sb[:, nt * DOUT:(nt + 1) * DOUT], h2p[:],
                        AF.Copy)

                prev = None
                for nt in range(NT):
                    st = e2_front(nt)
                    if prev is not None:
                        e2_back(nt - 1, prev)
                    prev = st
                e2_back(NT - 1, prev)
                # batched log: one table load for all tiles
                nc.scalar.activation(stat_sb[:, 2 * NT:3 * NT],
                                     stat_sb[:, NT:2 * NT], AF.Ln)
                if DEBUG_DUMP:
                    nc.sync.dma_start(out=DBG3[:], in_=h2p_sb[:])
                    nc.sync.dma_start(out=DBG4[:], in_=stat_sb[:])
                with tc.tile_pool(name="fin", bufs=3) as fp:
                    for nt in range(NT):
                        outt = fp.tile([P, DOUT], F32, tag="outt")
                        nc.vector.tensor_scalar(
                            out=outt[:],
                            in0=h2p_sb[:, nt * DOUT:(nt + 1) * DOUT],
                            scalar1=stat_sb[:, nt:nt + 1],
                            scalar2=stat_sb[:, 2 * NT + nt:2 * NT + nt + 1],
                            op0=OP.subtract, op1=OP.subtract)
                        nc.sync.dma_start(out=OUT[nt * P:(nt + 1) * P, :],
                                          in_=outt[:])

    nc.finalize()
    return nc


def preprocess(X, edge_index, edge_attr, W_heads, a_heads, W_out, a_out,
               NP=None):
    """Host-side index/weight preprocessing. Returns (in_maps, meta)."""
    import ml_dtypes
    N = X.shape[0]
    E = edge_index.shape[1]
    if NP is None:
        NP = ((N + N_CORES * P - 1) // (N_CORES * P)) * (N_CORES * P)
    NSH = NP // N_CORES
    NT = NSH // P
    NT_ALL = NP // P

    src = np.asarray(edge_index[0], dtype=np.int64)
    tgt = np.asarray(edge_index[1], dtype=np.int64)
    gt = src >> 7                       # global 128-node tile of src
    order = np.argsort(gt, kind="stable")
    cnt = np.bincount(gt, minlength=NP // P)
    ET = int(np.ceil(cnt.max() / P))
    TS = NT * ET

    starts = np.concatenate([[0], np.cumsum(cnt)])
    gs = gt[order]
    pos = np.arange(E) - starts[gs]
    core = gs // NT
    col = (gs % NT) * ET + pos // P
    lane = pos % P

    gidx = np.full((N_CORES, P, TS), NP, np.int32)
    gidx[core, lane, col] = tgt[order]
    # layer-2 table layout must match build_program's NCH (1 = rank-major)
    NCH = 1
    CH = (NT // NCH) * P
    tg = tgt[order]
    tg2 = ((tg % NSH) // CH) * (N_CORES * CH) + (tg // NSH) * CH + (tg % NSH) % CH
    gidx2 = np.full((N_CORES, P, TS), NP, np.int32)
    gidx2[core, lane, col] = tg2
    srcl = np.full((N_CORES, P, TS), -1.0, ml_dtypes.bfloat16)
    srcl[core, lane, col] = (src[order] & 127).astype(ml_dtypes.bfloat16)

    ea = np.asarray(edge_attr, np.float32)
    ah = np.asarray(a_heads, np.float32)
    ao = np.asarray(a_out, np.float32)
    se1_e = ea @ ah[:, 2 * DH:2 * DH + EA].T            # [E, H]
    se2_e = ea @ ao[2 * DOUT:2 * DOUT + EA]             # [E]
    se1 = np.zeros((N_CORES, NT, P, ET, H), ml_dtypes.bfloat16)
    se1[core, col // ET, lane, col % ET] = se1_e[order].astype(
        ml_dtypes.bfloat16)
    se1 = se1.reshape(N_CORES, NT, P, ET * H)
    se2 = np.zeros((N_CORES, NT, P, ET), ml_dtypes.bfloat16)
    se2[core, col // ET, lane, col % ET] = se2_e[order].astype(
        ml_dtypes.bfloat16)

    Xp = np.zeros((NP, DIN), np.float32)
    Xp[:N] = np.asarray(X, np.float32)
    XTT = np.ascontiguousarray(
        Xp.reshape(NT_ALL, P, DIN).transpose(0, 2, 1)).astype(
        ml_dtypes.bfloat16)
    # per-core combined stream: [own xto.T | se1 tile]
    XSE = np.concatenate(
        [XTT.reshape(N_CORES, NT, P, P), se1], axis=3)      # [8, NT, P, P+ET*H]

    Wh = np.asarray(W_heads, np.float32)
    Wo = np.asarray(W_out, np.float32)
    WC = np.ascontiguousarray(
        Wh.transpose(1, 0, 2).reshape(DIN, DC)).astype(ml_dtypes.bfloat16)
    # WA columns: [s_tgt (0:H) | s_src (H:2H)] to match the G1 row layout
    WA = np.concatenate([
        np.einsum("hkj,hj->kh", Wh, ah[:, DH:2 * DH]),
        np.einsum("hkj,hj->kh", Wh, ah[:, :DH])], axis=1).astype(
        ml_dtypes.bfloat16)
    # W2X columns per 128-row chunk: [h2 (128) | s_tgt2 | s_src2]
    base = np.concatenate(
        [Wo, (Wo @ ao[DOUT:2 * DOUT])[:, None], (Wo @ ao[:DOUT])[:, None]],
        axis=1).astype(np.float32)                      # [DC, G2WX]
    W2X = np.ascontiguousarray(
        base.reshape(4, P, G2WX).transpose(1, 0, 2).reshape(P, 4 * G2WX)
    ).astype(ml_dtypes.bfloat16)
    IOTF = np.tile(np.arange(P, dtype=np.float32)[None, :],
                   (P, ET)).astype(ml_dtypes.bfloat16)
    IDENT = np.eye(P, dtype=np.float32).astype(ml_dtypes.bfloat16)

    in_maps = []
    for c in range(N_CORES):
        in_maps.append({
            "xtt": XTT,
            "xse": XSE[c],
            "wc": WC, "wa": WA, "w2x": W2X,
            "iotf": IOTF, "ident": IDENT,
            "gidx": gidx[c], "gidx2": gidx2[c], "srcl": srcl[c],
            "se2": se2[c],
        })
    meta = dict(N=N, NP=NP, ET=ET)
    return in_maps, meta


def make_runner(nc, n_cores=N_CORES):
    """Build a reusable jitted SPMD executor for a finalized Bass module.

    Returns run(in_maps, n_iters=0) -> (per-core output dicts, seconds/iter).
    n_iters=0 executes once (timing NaN); n_iters>0 times that many extra
    executions with device-resident inputs.
    """
    import time
    import jax
    from jax.sharding import Mesh, PartitionSpec
    from jax.experimental.shard_map import shard_map
    from concourse import bass2jax
    from concourse.bass2jax import _bass_exec_p, partition_id_tensor

    bass2jax.install_neuronx_cc_hook()
    partition_name = nc.partition_id_tensor.name if nc.partition_id_tensor else None
    in_names, out_names, out_avals, zero_outs = [], [], [], []
    for alloc in nc.m.functions[0].allocations:
        if not isinstance(alloc, mybir.MemoryLocationSet):
            continue
        name = alloc.memorylocations[0].name
        if alloc.kind == "ExternalInput":
            if name != partition_name:
                in_names.append(name)
        elif alloc.kind == "ExternalOutput":
            out_names.append(name)
            shape = tuple(alloc.tensor_shape)
            dtype = mybir.dt.np(alloc.dtype)
            out_avals.append(jax.core.ShapedArray(shape, dtype))
            zero_outs.append(np.zeros(shape, dtype))
    n_params = len(in_names)
    all_in_names = list(in_names) + list(out_names)
    if partition_name is not None:
        all_in_names.append(partition_name)

    def _body(*args):
        operands = list(args)
        if partition_name is not None:
            operands.append(partition_id_tensor())
        outs = _bass_exec_p.bind(
            *operands,
            out_avals=tuple(out_avals),
            in_names=tuple(all_in_names),
            out_names=tuple(out_names),
            lowering_input_output_aliases=(),
            sim_require_finite=True,
            sim_require_nnan=True,
            nc=nc,
        )
        return tuple(outs)

    devices = jax.devices()[:n_cores]
    mesh = Mesh(np.asarray(devices), ("core",))
    in_specs = (PartitionSpec("core"),) * (n_params + len(out_names))
    out_specs = (PartitionSpec("core"),) * len(out_names)
    sharded = jax.jit(
        shard_map(_body, mesh=mesh, in_specs=in_specs, out_specs=out_specs,
                  check_rep=False),
        keep_unused=True,
    )

    def run(in_maps, n_iters=0):
        per_core = [[np.asarray(m[name]) for name in in_names] for m in in_maps]
        concat_in = [
            np.concatenate([per_core[c][i] for c in range(n_cores)], axis=0)
            for i in range(n_params)
        ]
        concat_zeros = [
            np.zeros((n_cores * z.shape[0], *z.shape[1:]), z.dtype)
            for z in zero_outs
        ]
        args = [jax.device_put(a) for a in concat_in]
        args += [jax.device_put(a) for a in concat_zeros]
        out = sharded(*args)
        jax.block_until_ready(out)
        dt = float("nan")
        if n_iters:
            t0 = time.perf_counter()
            for _ in range(n_iters):
                out = sharded(*args)
                jax.block_until_ready(out)
            dt = (time.perf_counter() - t0) / n_iters
        results = [
            {
                name: np.asarray(out[i]).reshape(n_cores, *out_avals[i].shape)[c]
                for i, name in enumerate(out_names)
            }
            for c in range(n_cores)
        ]
        return results, dt

    return run


_RUNNER_CACHE = {}


def _get_runner(NP, ET, repeat=1):
    key = (NP, ET, repeat)
    if key not in _RUNNER_CACHE:
        nc = build_program(NP, ET, repeat=repeat)
        _RUNNER_CACHE[key] = make_runner(nc, N_CORES)
    return _RUNNER_CACHE[key]


def kernel(X, edge_index, edge_attr, W_heads, a_heads, W_out, a_out):
    in_maps, meta = preprocess(X, edge_index, edge_attr, W_heads, a_heads,
                               W_out, a_out)
    run = _get_runner(meta["NP"], meta["ET"])
    results, _ = run(in_maps, n_iters=0)
    out = np.concatenate([results[c]["out"] for c in range(N_CORES)], axis=0)
    return out[:meta["N"]].astype(np.float32)
